# revision 37
# baseline (speedup 1.0000x reference)
"""GraphSAGE 2-layer (SAGEConv mean-aggregation) Bass kernel for 8 TRN2 NeuronCores.

Strategy (see spec sharding_hint):
  - Destination nodes sharded across 8 cores (12500/core). Within each core a
    greedy balancer assigns nodes to 98 windows x 128 slots so that each
    (window, src-block) cell has <= ~512 edges -> near-uniform SPMD schedule.
  - Edges partitioned by destination core, sorted by (window, src-block) and
    by source row within each cell (HBM locality for the gathers).
  - Aggregation: dma_gather pulls rows (bf16) from HBM in 4 source blocks
    (int16 index limit); selection tiles (pure 0/1 one-hot, [128 edges x 128
    slots]) are streamed pre-built from HBM in fp8 (half the bytes of bf16,
    exactly representable); TensorE accumulates raw neighbor sums^T per
    window in PSUM. The mean division (invdeg per destination slot) is
    applied by one DVE tensor_tensor(mult) per window against an SBUF-
    resident [128, SLOTS] broadcast tile of per-slot inverse degrees.
  - Transform per window: two 128x128 matmuls (+ bias) produce hT; layer-1 h
    is transposed to row-major SLOT order and written with plain DMA (no
    scatter). One AllGather of the slot-ordered shard forms hfull; layer-2
    gathers reference (core, slot) coordinates directly - 2*NSH == BS so
    layer-2 blocks coincide with layer-1 source blocks and the cell
    structure is shared.
  - Final layer-2 output is written in slot order (bf16) and inverse-
    permuted on host.
"""

import sys

sys.path.insert(0, "/opt/trn_rl_repo")

from contextlib import ExitStack
from dataclasses import dataclass

import ml_dtypes
import numpy as np

import concourse.bacc as bacc
import concourse.bass as bass
import concourse.mybir as mybir
import concourse.tile as tile
from concourse.bass_utils import run_bass_kernel_spmd

BF = mybir.dt.bfloat16
F32 = mybir.dt.float32
I16 = mybir.dt.int16
bfnp = ml_dtypes.bfloat16
f8np = ml_dtypes.float8_e4m3
FP8 = mybir.dt.float8e4


@dataclass
class Cfg:
    N: int = 100000      # total nodes
    D: int = 128         # feature dim
    C: int = 8           # cores
    NB: int = 4          # source blocks (int16 gather index limit)
    WN: int = 98         # windows per core (128 dst nodes each)
    CALL: int = 1024     # gather indices per dma_gather call
    SCW: int = 4         # windows per h-store dma
    OCW: int = 4         # windows per final output dma
    G: int = 1           # AllGather chunks (window groups)
    DVE_EVERY: int = 0   # 1 of every DVE_EVERY sel-tiles built on DVE (0: none)
    ACT_EVERY: int = 0   # 1 of every ACT_EVERY sel-tiles built on ScalarE (0: none)

    @property
    def NSH(self):
        return self.N // self.C

    @property
    def BS(self):
        return self.N // self.NB

    @property
    def SLOTS(self):
        return self.WN * 128

    @property
    def CALLCH(self):
        return self.CALL // 128

    @property
    def WCH(self):
        # window group boundaries for AllGather chunks
        per = -(-self.WN // self.G)
        return [min(self.WN, per * g) for g in range(self.G + 1)]


CFG = Cfg()


# ---------------------------------------------------------------- host prep


def _balance_core(dnb, WN, cap=128, ctarget=512):
    """Assign nodes (rows of dnb, per-block in-degree vectors) to WN bins of
    <=cap nodes, aiming for per-(bin, block) sums <= target. Overflow (when a
    block's total exceeds WN*ctarget) is concentrated in the LAST windows.
    Returns (bin id per node, binsum)."""
    nn, NB = dnb.shape
    T = dnb.sum(0)
    # per-block overflow chunks, assigned to tail windows
    target = np.full((WN, NB), ctarget, np.int64)
    for b in range(NB):
        q = max(0, -(-int(T[b] - WN * ctarget) // 128))
        for i in range(min(q, WN)):
            target[WN - 1 - i, b] += 128
    tot = dnb.sum(1)
    order = np.argsort(-tot, kind="stable")
    binsum = np.zeros((WN, NB), np.int64)
    binslots = np.zeros(WN, np.int64)
    assign = np.full(nn, -1, np.int64)
    tgt = target.astype(np.float64)
    for n in order:
        dv = dnb[n]
        fill = ((binsum + dv) / tgt).max(axis=1)
        fill += 1e-5 * binslots
        fill[binslots >= cap] = 1e30
        j = int(np.argmin(fill))
        assign[n] = j
        binsum[j] += dv
        binslots[j] += 1

    # repair: evict small-degree nodes from violated cells into bins with
    # slack (move if a slot is free, else swap with a light partner)
    for _ in range(30):
        viol = np.argwhere(binsum > target)
        if len(viol) == 0:
            break
        moved = 0
        for j, b in viol:
            guard = 0
            while binsum[j, b] > target[j, b] and guard < 64:
                guard += 1
                members = np.where(assign == j)[0]
                mb = dnb[members, b]
                cand_n = members[mb > 0]
                if len(cand_n) == 0:
                    break
                # smallest positive contribution first
                cand_n = cand_n[np.argsort(dnb[cand_n, b], kind="stable")]
                done = False
                for n in cand_n[:8]:
                    dv = dnb[n]
                    ok = ((binsum + dv) <= target).all(axis=1) & (binslots < cap)
                    ok[j] = False
                    cand = np.where(ok)[0]
                    if len(cand):
                        j2 = int(cand[np.argmin(((binsum[cand] + dv) / target[cand]).max(1))])
                        assign[n] = j2
                        binsum[j] -= dv
                        binsum[j2] += dv
                        binslots[j] -= 1
                        binslots[j2] += 1
                        moved += 1
                        done = True
                        break
                    # swap with the lightest partner in low-fill bins
                    for j2 in np.argsort(binsum[:, b])[:24]:
                        if j2 == j:
                            continue
                        mem2 = np.where(assign == j2)[0]
                        if len(mem2) == 0:
                            continue
                        m = mem2[np.argmin(dnb[mem2, b])]
                        dm = dnb[m]
                        if dm[b] >= dv[b]:
                            continue
                        nj = binsum[j] - dv + dm
                        nj2 = binsum[j2] - dm + dv
                        if (nj <= target[j]).all() and (nj2 <= target[j2]).all():
                            assign[n], assign[m] = j2, j
                            binsum[j] = nj
                            binsum[j2] = nj2
                            moved += 1
                            done = True
                            break
                    if done:
                        break
                if not done:
                    break
        if moved == 0:
            break
    return assign, binsum


def _layer_sched(counts, cfg):
    """counts: [C, WN, NBL] per-core cell sizes -> shared schedule dict."""
    K = np.ceil(counts / 128).astype(np.int64).max(axis=0)  # [WN, NBL]
    TCH = int(K.sum())
    Sb = (K.sum(axis=0) * 128).astype(np.int64)             # [NBL] idx slots
    ncalls = np.ceil(Sb / cfg.CALL).astype(np.int64)
    lastvalid = Sb - (ncalls - 1) * cfg.CALL
    # DVE / streamed-sval split, round-robin by chunk column
    if cfg.DVE_EVERY > 0:
        is_dve = (np.arange(TCH) % cfg.DVE_EVERY) == 0
    else:
        is_dve = np.zeros(TCH, bool)
    if cfg.ACT_EVERY > 0:
        is_act = (~is_dve) & ((np.arange(TCH) % cfg.ACT_EVERY) == 1)
    else:
        is_act = np.zeros(TCH, bool)
    is_dve = is_dve | is_act                 # "on-chip" tiles (drel-driven)
    dcol_of = np.cumsum(is_dve) - 1          # on-chip drel column index
    svi_of = np.cumsum(~is_dve) - 1          # streamed sval tile index
    NDV = int(is_dve.sum())
    NSV = TCH - NDV
    NBL = K.shape[1]
    GCOLS_B = [int(ncalls[b]) * (cfg.CALL // 16) for b in range(NBL)]
    GOFF = np.concatenate([[0], np.cumsum(GCOLS_B)]).astype(int)
    return dict(K=K, TCH=TCH, Sb=Sb, ncalls=ncalls, lastvalid=lastvalid,
                is_dve=is_dve, is_act=is_act, dcol_of=dcol_of, svi_of=svi_of,
                NDV=NDV, NSV=max(NSV, 1), GOFF=GOFF)


def _core_layer_maps(sch, cellcnt, rel_src, ep, ev, cfg):
    """Build gather idx stream + per-chunk metadata for one (core, layer).
    rel_src: int64 block-relative row index per edge (cell-sorted order).
    Returns dict with gidx [128, GCOLS], drel/vval [128, NDV] f32,
    sval [128, NSV*128] bf16."""
    WN, CALL = cfg.WN, cfg.CALL
    K, TCH, Sb, ncalls = sch["K"], sch["TCH"], sch["Sb"], sch["ncalls"]
    is_dve, dcol_of, svi_of = sch["is_dve"], sch["dcol_of"], sch["svi_of"]
    NDV, NSV = sch["NDV"], sch["NSV"]
    NBL = K.shape[1]

    gstreams = [np.zeros(int(ncalls[b]) * CALL, np.int16) for b in range(NBL)]
    for b in range(NBL):
        if Sb[b] < ncalls[b] * CALL:
            gstreams[b][Sb[b]:] = -1  # tail of last call: skipped by DMA
    drel = np.full((max(NDV, 1), 128), -255.0, np.float32)
    vval = np.zeros((max(NDV, 1), 128), np.float32)
    sval = np.zeros((NSV, 128, 128), f8np)

    eoff = 0
    gcol = 0
    posb = np.zeros(NBL, np.int64)
    for w in range(WN):
        for b in range(NBL):
            L = int(cellcnt[w, b])
            kwb = int(K[w, b])
            if kwb == 0:
                assert L == 0
                continue
            sl = slice(eoff, eoff + L)
            st = int(posb[b])
            gstreams[b][st:st + L] = rel_src[sl].astype(np.int16)
            epc = ep[sl]
            evc = ev[sl]
            for kk in range(kwb):
                r0, r1 = kk * 128, min((kk + 1) * 128, L)
                gc = gcol + kk
                if is_dve[gc]:
                    dc = int(dcol_of[gc])
                    if r1 > r0:
                        drel[dc, :r1 - r0] = -epc[r0:r1]
                        vval[dc, :r1 - r0] = evc[r0:r1].astype(np.float32)
                else:
                    si = int(svi_of[gc])
                    if r1 > r0:
                        sval[si, np.arange(r1 - r0), epc[r0:r1]] = 1.0
            posb[b] += kwb * 128
            gcol += kwb
            eoff += L
    assert gcol == TCH

    gparts = []
    for b in range(NBL):
        arr = gstreams[b].reshape(-1, 16).T  # [16, Sb_pad/16]
        gparts.append(arr)
    gidx16 = np.concatenate(gparts, axis=1)          # [16, GCOLS]
    gidx = np.tile(gidx16, (8, 1)).astype(np.int16)  # [128, GCOLS]
    return dict(
        gidx=gidx,
        drel=np.ascontiguousarray(drel.T), vval=np.ascontiguousarray(vval.T),
        sval=np.ascontiguousarray(sval.transpose(1, 0, 2).reshape(128, -1)),
    ), eoff


def prep(x, edge_index, cfg=CFG):
    """Host-side sharding/schedule. Returns (schedule, per-core input maps)."""
    C, NB, WN, NSH, BS = cfg.C, cfg.NB, cfg.WN, cfg.NSH, cfg.BS
    SLOTS, G = cfg.SLOTS, cfg.G
    WCH = cfg.WCH
    src = np.asarray(edge_index[0]).astype(np.int64)
    dst = np.asarray(edge_index[1]).astype(np.int64)

    deg = np.bincount(dst, minlength=cfg.N).astype(np.float64)
    invdeg = (1.0 / np.maximum(deg, 1.0)).astype(np.float32)
    vedge_all = invdeg[dst].astype(bfnp)

    ecore = dst // NSH
    eblock1 = src // BS

    # --- per-core balance: node-local id -> (window, pos)
    win_of = np.zeros(cfg.N, np.int64)   # window within core
    pos_of = np.zeros(cfg.N, np.int64)   # slot within window
    counts1 = np.zeros((C, WN, NB), np.int64)
    for c in range(C):
        lo = c * NSH
        dnb = np.zeros((NSH, NB), np.int64)
        emask = ecore == c
        np.add.at(dnb, (dst[emask] - lo, eblock1[emask]), 1)
        assign, binsum = _balance_core(dnb, WN)
        # order bins by descending per-block chunk tuple so heavy cells align
        # at the same window index across cores
        kt = np.ceil(binsum / 128).astype(np.int64)
        key = [tuple(-kt[j]) + tuple(-binsum[j]) for j in range(WN)]
        order = sorted(range(WN), key=lambda j: key[j])
        rank = np.empty(WN, np.int64)
        rank[order] = np.arange(WN)
        w = rank[assign]
        win_of[lo:lo + NSH] = w
        # position within window: stable by node id
        order2 = np.lexsort((np.arange(NSH), w))
        pos = np.zeros(NSH, np.int64)
        pcount = np.zeros(WN, np.int64)
        for m in order2:
            pos[m] = pcount[w[m]]
            pcount[w[m]] += 1
        pos_of[lo:lo + NSH] = pos
        cnt = np.zeros((WN, NB), np.int64)
        np.add.at(cnt, (w[dst[emask] - lo], eblock1[emask]), 1)
        counts1[c] = cnt

    # layer-2 geometry: hfull is chunk-major: for AllGather chunk g
    # (windows [WCH[g], WCH[g+1])), node row = cbase[g] + core*crows[g] +
    # (win - WCH[g])*128 + pos; gather block g covers rows
    # [cbase[g], cbase[g+1]).
    if G == 1:
        # core-major hfull: blocks of 2 cores (2*NSH == BS) share layer-1's
        # cell structure exactly
        g2_of = (np.arange(cfg.N) // NSH) * SLOTS + win_of * 128 + pos_of
        eblock2 = eblock1
        cbase = np.arange(NB + 1, dtype=np.int64) * 2 * SLOTS
        counts2 = counts1
    else:
        crows = np.array([(WCH[g + 1] - WCH[g]) * 128 for g in range(G)], np.int64)
        cbase = np.concatenate([[0], np.cumsum(crows * C)]).astype(np.int64)
        for g in range(G):
            assert crows[g] * C <= 32767, (g, crows[g] * C)
        gchunk_of_w = np.zeros(WN, np.int64)
        for g in range(G):
            gchunk_of_w[WCH[g]:WCH[g + 1]] = g
        wch_arr = np.asarray(WCH)
        vg = gchunk_of_w[win_of]
        g2_of = (cbase[vg] + (np.arange(cfg.N) // NSH) * crows[vg]
                 + (win_of - wch_arr[vg]) * 128 + pos_of)
        eblock2 = vg[src]
        counts2 = np.zeros((C, WN, G), np.int64)
        for c in range(C):
            emask = ecore == c
            np.add.at(counts2[c], (win_of[dst[emask]], eblock2[emask]), 1)

    sch1 = _layer_sched(counts1, cfg)
    sch2 = _layer_sched(counts2, cfg)

    ewin = win_of[dst]
    epos = pos_of[dst]

    in_maps = []
    for c in range(C):
        lo = c * NSH
        emask = ecore == c
        es = src[emask]
        ew, eb1, eb2 = ewin[emask], eblock1[emask], eblock2[emask]
        ep = epos[emask]
        ev = vedge_all[emask]
        eg2 = g2_of[es]

        # layer 1: cells (w, src//BS), sorted by src within cell
        o1 = np.lexsort((es, eb1, ew))
        m1, ne1 = _core_layer_maps(
            sch1, counts1[c], (es - eb1 * BS)[o1], ep[o1], ev[o1], cfg)
        assert ne1 == es.shape[0]
        # layer 2: cells (w, chunk(src)), sorted by hfull row within cell
        rel2 = eg2 - cbase[eb2]
        o2 = np.lexsort((rel2, eb2, ew))
        m2, ne2 = _core_layer_maps(
            sch2, counts2[c], rel2[o2], ep[o2], ev[o2], cfg)
        assert ne2 == es.shape[0]

        # slot s = win*128+pos of node-local rows; dummy slots unused
        msk = np.arange(cfg.N)[lo:lo + NSH]
        sl_idx = win_of[msk] * 128 + pos_of[msk]
        sl_nodes = np.full(cfg.SLOTS, -1, np.int64)
        sl_nodes[sl_idx] = np.arange(NSH)

        # xT in slot order
        xT = np.zeros((cfg.D, cfg.SLOTS), bfnp)
        xT[:, sl_idx] = np.asarray(x[lo:lo + NSH]).astype(bfnp).T
        invsl = np.ones((cfg.SLOTS,), np.float32)
        invsl[sl_idx] = invdeg[lo:lo + NSH]
        bcast = np.ascontiguousarray(
            np.broadcast_to(invsl.astype(bfnp), (128, cfg.SLOTS)))

        in_maps.append(dict(
            gidx1=m1["gidx"], dstrel1=m1["drel"], vval1=m1["vval"], sval1=m1["sval"],
            gidx2=m2["gidx"], dstrel2=m2["drel"], vval2=m2["vval"], sval2=m2["sval"],
            xT=np.ascontiguousarray(xT), bcast=bcast,
            slot_nodes=sl_nodes,                   # host-only
        ))

    sched = dict(sch1=sch1, sch2=sch2, cbase=cbase,
                 TCH=sch1["TCH"] + sch2["TCH"])
    return sched, in_maps


# ---------------------------------------------------------------- program


def build(cfg, sched):
    sch = {1: sched["sch1"], 2: sched["sch2"]}
    cbase = sched["cbase"]
    C, D, NB, WN, BS = cfg.C, cfg.D, cfg.NB, cfg.WN, cfg.BS
    CALL, CALLCH, SLOTS, G = cfg.CALL, cfg.CALLCH, cfg.SLOTS, cfg.G
    WCH = cfg.WCH

    nc = bacc.Bacc(None, num_devices=C, num_swdge_queues=4,
                   dynamic_dma_scratch_size=24576)
    x_d = nc.dram_tensor("xbf", [cfg.N, D], BF, kind="ExternalInput")
    xT_d = nc.dram_tensor("xT", [D, SLOTS], BF, kind="ExternalInput")
    gidx_d, drel_d, vval_d, sval_d = {}, {}, {}, {}
    for L in (1, 2):
        s = sch[L]
        gidx_d[L] = nc.dram_tensor(f"gidx{L}", [128, int(s["GOFF"][-1])], I16,
                                   kind="ExternalInput")
        drel_d[L] = nc.dram_tensor(f"dstrel{L}", [128, max(s["NDV"], 1)], F32,
                                   kind="ExternalInput")
        vval_d[L] = nc.dram_tensor(f"vval{L}", [128, max(s["NDV"], 1)], F32,
                                   kind="ExternalInput")
        sval_d[L] = nc.dram_tensor(f"sval{L}", [128, s["NSV"] * 128], FP8,
                                   kind="ExternalInput")
    w_d = {}
    for nm in ("wlt1", "wrt1", "wlt2", "wrt2"):
        w_d[nm] = nc.dram_tensor(nm, [D, D], BF, kind="ExternalInput")
    b1_d = nc.dram_tensor("b1c", [D, 1], F32, kind="ExternalInput")
    b2_d = nc.dram_tensor("b2r", [1, D], F32, kind="ExternalInput")
    bc_d = nc.dram_tensor("bcast", [128, SLOTS], BF, kind="ExternalInput")
    out_d = nc.dram_tensor("out", [SLOTS, D], BF, kind="ExternalOutput")

    ident_d = nc.inline_tensor(np.eye(128, dtype=bfnp), "identc")
    iota_d = nc.inline_tensor(
        np.broadcast_to(np.arange(128, dtype=bfnp), (128, 128)).copy(), "iotac")
    ones_d = nc.inline_tensor(np.ones((1, 128), np.float32), "onesc")

    hsl_d = nc.dram_tensor("hslots", [SLOTS, D], BF)  # Internal, slot order
    hfull_d = nc.dram_tensor("hfull", [C * SLOTS, D], BF, addr_space="Shared")

    with tile.TileContext(nc) as tc, ExitStack() as ctx:
        const = ctx.enter_context(tc.tile_pool(name="const", bufs=1))
        meta = ctx.enter_context(tc.tile_pool(name="meta", bufs=1))
        gpool = ctx.enter_context(tc.tile_pool(name="gather", bufs=5))
        spool = ctx.enter_context(tc.tile_pool(name="sv", bufs=6))
        ohp = ctx.enter_context(tc.tile_pool(name="oh", bufs=1))
        dhp = ctx.enter_context(tc.tile_pool(name="dh", bufs=1))
        mwp = ctx.enter_context(tc.tile_pool(name="mw", bufs=6))
        htp = ctx.enter_context(tc.tile_pool(name="ht", bufs=1))
        xtp = ctx.enter_context(tc.tile_pool(name="xt", bufs=1))
        stgp = ctx.enter_context(tc.tile_pool(name="stg", bufs=2))
        ostgp = ctx.enter_context(tc.tile_pool(name="ostg", bufs=2))
        psA = ctx.enter_context(tc.tile_pool(name="psA", bufs=4, space="PSUM"))
        psB = ctx.enter_context(tc.tile_pool(name="psB", bufs=2, space="PSUM"))
        psT = ctx.enter_context(tc.tile_pool(name="psT", bufs=2, space="PSUM"))
        bcp = ctx.enter_context(tc.tile_pool(name="bc", bufs=1))

        def load(pool, dram, shape, dtype):
            t = pool.tile(shape, dtype, tag=dram.name)
            nc.sync.dma_start(t[:], dram[:])
            return t

        ident_s = load(const, ident_d, [128, 128], BF)
        iota_s = load(const, iota_d, [128, 128], BF)
        ones_s = load(const, ones_d, [1, 128], F32)
        w_s = {nm: load(const, w_d[nm], [D, D], BF) for nm in w_d}
        b1_s = load(const, b1_d, [D, 1], F32)
        b2_s = load(const, b2_d, [1, D], F32)

        drel_s = {L: load(meta, drel_d[L], [128, max(sch[L]["NDV"], 1)], F32)
                  for L in (1, 2)}
        vval_s = {L: load(meta, vval_d[L], [128, max(sch[L]["NDV"], 1)], F32)
                  for L in (1, 2)}
        gidx_s = {}
        for L in (1, 2):
            GOFF = sch[L]["GOFF"]
            NBL = sch[L]["K"].shape[1]
            gidx_s[L] = meta.tile([128, int(GOFF[-1])], I16, tag=f"gidx{L}",
                                  name=f"gidx{L}")
            for _b in range(NBL):
                nc.scalar.dma_start(gidx_s[L][:, int(GOFF[_b]):int(GOFF[_b + 1])],
                                    gidx_d[L][:, int(GOFF[_b]):int(GOFF[_b + 1])])
        xT_s = xtp.tile([D, SLOTS], BF, tag="xT", name="xT_s")
        nc.scalar.dma_start(xT_s[:], xT_d[:])
        bc_s = bcp.tile([128, SLOTS], BF, tag="bcast", name="bc_s")
        nc.scalar.dma_start(bc_s[:], bc_d[:])
        hT_s = htp.tile([D, SLOTS], BF, tag="hT")

        def run_layer(L, w_lo, w_hi):
            s = sch[L]
            K, ncalls, lastvalid = s["K"], s["ncalls"], s["lastvalid"]
            is_dve, dcol_of, svi_of, NSV = (s["is_dve"], s["dcol_of"],
                                            s["svi_of"], s["NSV"])
            GOFF = s["GOFF"]
            NBL = K.shape[1]
            src_d = x_d if L == 1 else hfull_d
            drl, vvl, gix = drel_s[L], vval_s[L], gidx_s[L]
            st_ = run_layer.state.setdefault(
                L, dict(posb=[0] * NBL, gt=[None] * NBL, gcol=0, stile=None,
                        stg=None, ostg=None, bct=None))
            posb, gt = st_["posb"], st_["gt"]
            SCW, OCW = cfg.SCW, cfg.OCW
            for w in range(w_lo, w_hi):
                nchunks_w = int(K[w].sum())
                psum_a = psA.tile([128, 128], F32, tag="agg", name=f"agg{L}_{w}")
                ci = 0
                for b in range(NBL):
                    for k in range(int(K[w, b])):
                        pos = posb[b]
                        call_i, col = divmod(pos, CALLCH)
                        if col == 0:
                            gt[b] = gpool.tile([128, CALLCH, 128], BF, tag=f"g{b}",
                                               name=f"g{b}_{L}_{call_i}")
                            nvalid = CALL if call_i < int(ncalls[b]) - 1 else int(lastvalid[b])
                            ioff = GOFF[b] + call_i * (CALL // 16)
                            if L == 1:
                                in_ap = src_d[b * BS:(b + 1) * BS, :]
                            else:
                                in_ap = src_d[int(cbase[b]):int(cbase[b + 1]), :]
                            nc.gpsimd.dma_gather(
                                out_ap=gt[b][:],
                                in_ap=in_ap,
                                idxs_ap=gix[:, ioff:ioff + CALL // 16],
                                num_idxs=CALL,
                                num_idxs_reg=nvalid,
                                elem_size=D,
                            )
                        gcol = st_["gcol"]
                        if is_dve[gcol]:
                            dc = int(dcol_of[gcol])
                            dd = dhp.tile([128, 128], BF, tag="dh",
                                          name=f"dh{L}_{gcol}")
                            nc.scalar.activation(
                                dd[:], iota_s[:],
                                mybir.ActivationFunctionType.Abs,
                                bias=drl[:, dc:dc + 1], scale=1.0)
                            S = ohp.tile([128, 128], BF, tag="oh",
                                         name=f"oh{L}_{gcol}")
                            nc.scalar.activation(
                                S[:], dd[:],
                                mybir.ActivationFunctionType.Relu,
                                bias=1.0, scale=-1.0)
                            rhs_ap = S[:]
                        else:
                            si, sc = divmod(int(svi_of[gcol]), 8)
                            if sc == 0:
                                nch = min(8, NSV - si * 8)
                                stt = spool.tile([128, 8, 128], FP8, tag="sv",
                                                 name=f"sv{L}_{si}")
                                nc.sync.dma_start(
                                    stt[:, :nch, :],
                                    sval_d[L][:, si * 1024:si * 1024 + nch * 128])
                                st_["stile"] = stt
                            rhs_ap = st_["stile"][:, sc, :]
                        nc.tensor.matmul(
                            out=psum_a[:], lhsT=gt[b][:, col, :], rhs=rhs_ap,
                            start=(ci == 0), stop=(ci == nchunks_w - 1),
                        )
                        st_["gcol"] += 1
                        posb[b] += 1
                        ci += 1
                m_s = mwp.tile([128, 128], BF, tag="mw", name=f"mw{L}_{w}")
                wsl = slice(w * 128, (w + 1) * 128)
                if nchunks_w:
                    nc.vector.tensor_tensor(out=m_s[:], in0=psum_a[:],
                                            in1=bc_s[:, wsl],
                                            op=mybir.AluOpType.mult)
                else:
                    nc.vector.memset(m_s[:], 0.0)
                if L == 1:
                    psum_h = psB.tile([128, 128], F32, tag="h", name=f"h{L}_{w}")
                    nc.tensor.matmul(out=psum_h[:], lhsT=w_s["wlt1"][:], rhs=m_s[:],
                                     start=True, stop=False)
                    nc.tensor.matmul(out=psum_h[:], lhsT=w_s["wrt1"][:],
                                     rhs=xT_s[:, wsl], start=False, stop=True)
                    nc.scalar.activation(hT_s[:, wsl], psum_h[:],
                                         mybir.ActivationFunctionType.Identity,
                                         bias=b1_s[:, 0:1], scale=1.0)
                    psum_t = psT.tile([128, 128], BF, tag="tr", name=f"tr{w}")
                    nc.tensor.transpose(psum_t[:], hT_s[:, wsl], ident_s[:])
                    wi = (w - w_lo) % SCW
                    if wi == 0:
                        st_["stg"] = stgp.tile([128, SCW, 128], BF, tag="stg",
                                               name=f"stg{w}")
                    nc.scalar.copy(st_["stg"][:, wi, :], psum_t[:])
                    if wi == SCW - 1 or w == w_hi - 1:
                        used = wi + 1
                        w0 = w - wi
                        hap = hsl_d[:].rearrange("(w p) f -> p w f", p=128)
                        nc.sync.dma_start(hap[:, w0:w0 + used, :],
                                          st_["stg"][:, :used, :])
                else:
                    psum_h = psB.tile([128, 128], F32, tag="h", name=f"h{L}_{w}")
                    nc.tensor.matmul(out=psum_h[:], lhsT=m_s[:], rhs=w_s["wlt2"][:],
                                     start=True, stop=False)
                    nc.tensor.matmul(out=psum_h[:], lhsT=hT_s[:, wsl],
                                     rhs=w_s["wrt2"][:], start=False, stop=False)
                    nc.tensor.matmul(out=psum_h[:], lhsT=ones_s[0:1, :],
                                     rhs=b2_s[0:1, :], start=False, stop=True)
                    wi = w % OCW
                    if wi == 0:
                        st_["ostg"] = ostgp.tile([128, OCW, 128], BF, tag="ostg",
                                                 name=f"ostg{w}")
                    nc.scalar.copy(st_["ostg"][:, wi, :], psum_h[:])
                    if wi == OCW - 1 or w == WN - 1:
                        used = wi + 1
                        w0 = w - wi
                        oap = out_d[:].rearrange("(w p) f -> p w f", p=128)
                        nc.sync.dma_start(oap[:, w0:w0 + used, :],
                                          st_["ostg"][:, :used, :])

        run_layer.state = {}
        # layer 1 in window chunks; AllGather each chunk as soon as its
        # windows are stored (overlaps the collective with remaining compute)
        for g in range(G):
            run_layer(1, WCH[g], WCH[g + 1])
            r0, r1 = WCH[g] * 128, WCH[g + 1] * 128
            out_lo = 0 if G == 1 else int(cbase[g])
            out_hi = C * SLOTS if G == 1 else int(cbase[g + 1])
            nc.gpsimd.collective_compute(
                "AllGather", mybir.AluOpType.bypass,
                replica_groups=[list(range(C))],
                ins=[hsl_d[r0:r1, :]],
                outs=[hfull_d[out_lo:out_hi, :]],
            )
        run_layer(2, 0, WN)

    # spread SWDGE gather descriptor generation across the 4 SWDGE queues
    # (parallel Q7 pairs). Tile assigned DMASW lanes round-robin in scheduled
    # order; keep sem-lane <-> queue binding consistent by deriving the queue
    # from the lane (lane % 4).
    from concourse.tile_sem_assignment import PROC_NAME_TO_IDX
    dmasw0 = PROC_NAME_TO_IDX["DMASW0"]
    for inst in nc.inst_map.values():
        if isinstance(inst, (mybir.InstDMAGatherAnt, mybir.InstDMAScatterAddAnt)):
            proc = getattr(inst, "bass_scheduled_proc", None)
            if proc is not None and dmasw0 <= proc < dmasw0 + 8:
                inst.queue_num = (proc - dmasw0) % 4

    nc.compile()
    return nc


# ---------------------------------------------------------------- kernel


def kernel(**inputs):
    cfg = CFG
    x = np.asarray(inputs["x"], np.float32)
    ei = np.asarray(inputs["edge_index"])
    sched, in_maps = prep(x, ei, cfg)
    nc = build(cfg, sched)

    x_bf = x.astype(bfnp)
    shared = dict(
        xbf=x_bf,
        wlt1=np.ascontiguousarray(np.asarray(inputs["Wl1"], np.float32).T.astype(bfnp)),
        wrt1=np.ascontiguousarray(np.asarray(inputs["Wr1"], np.float32).T.astype(bfnp)),
        wlt2=np.ascontiguousarray(np.asarray(inputs["Wl2"], np.float32).T.astype(bfnp)),
        wrt2=np.ascontiguousarray(np.asarray(inputs["Wr2"], np.float32).T.astype(bfnp)),
        b1c=np.asarray(inputs["b1"], np.float32).reshape(cfg.D, 1).copy(),
        b2r=np.asarray(inputs["b2"], np.float32).reshape(1, cfg.D).copy(),
    )
    slot_nodes = [m.pop("slot_nodes") for m in in_maps]
    run_maps = [dict(shared, **{k: v for k, v in m.items()}) for m in in_maps]

    res = None
    last_err = None
    for attempt in range(3):
        try:
            res = run_bass_kernel_spmd(nc, run_maps, core_ids=list(range(cfg.C)))
            break
        except Exception as e:  # transient device wedge: retry
            last_err = e
            import time
            time.sleep(10)
    if res is None:
        raise last_err
    out = np.empty((cfg.N, cfg.D), np.float32)
    for c in range(cfg.C):
        oc = res.results[c]["out"]
        sn = slot_nodes[c]
        real = sn >= 0
        out[c * cfg.NSH + sn[real]] = oc[real]
    return out


if __name__ == "__main__":
    d = np.load("/tmp/inputs.npz")
    ins = {k: d[k] for k in ("x", "edge_index", "Wl1", "Wr1", "b1", "Wl2", "Wr2", "b2")}
    got = kernel(**ins)
    exp = d["expected"]
    err = np.abs(got - exp).max() / np.abs(exp).max()
    print("Relative error:", err)


# revision 38
# speedup vs baseline: 1.0599x; 1.0599x over previous
"""GraphSAGE 2-layer (SAGEConv mean-aggregation) Bass kernel for 8 TRN2 NeuronCores.

Strategy (see spec sharding_hint):
  - Destination nodes sharded across 8 cores (12500/core). Within each core a
    greedy balancer assigns nodes to 98 windows x 128 slots so that each
    (window, src-block) cell has <= ~512 edges -> near-uniform SPMD schedule.
  - Edges partitioned by destination core, sorted by (window, src-block) and
    by source row within each cell (HBM locality for the gathers).
  - Aggregation: dma_gather pulls rows (bf16) from HBM in 4 source blocks
    (int16 index limit); selection tiles (pure 0/1 one-hot, [128 edges x 128
    slots]) are streamed pre-built from HBM in fp8 (half the bytes of bf16,
    exactly representable); TensorE accumulates raw neighbor sums^T per
    window in PSUM. The mean division (invdeg per destination slot) is
    applied by one DVE tensor_tensor(mult) per window against an SBUF-
    resident [128, SLOTS] broadcast tile of per-slot inverse degrees.
  - Transform per window: two 128x128 matmuls (+ bias) produce hT; layer-1 h
    is transposed to row-major SLOT order and written with plain DMA (no
    scatter). One AllGather of the slot-ordered shard forms hfull; layer-2
    gathers reference (core, slot) coordinates directly - 2*NSH == BS so
    layer-2 blocks coincide with layer-1 source blocks and the cell
    structure is shared.
  - Final layer-2 output is written in slot order (bf16) and inverse-
    permuted on host.
"""

import sys

sys.path.insert(0, "/opt/trn_rl_repo")

from contextlib import ExitStack
from dataclasses import dataclass

import ml_dtypes
import numpy as np

import concourse.bacc as bacc
import concourse.bass as bass
import concourse.mybir as mybir
import concourse.tile as tile
from concourse.bass_utils import run_bass_kernel_spmd

BF = mybir.dt.bfloat16
F32 = mybir.dt.float32
I16 = mybir.dt.int16
bfnp = ml_dtypes.bfloat16
f8np = ml_dtypes.float8_e4m3
FP8 = mybir.dt.float8e4


@dataclass
class Cfg:
    N: int = 100000      # total nodes
    D: int = 128         # feature dim
    C: int = 8           # cores
    NB: int = 4          # source blocks (int16 gather index limit)
    WN: int = 98         # windows per core (128 dst nodes each)
    CALL: int = 1024     # gather indices per dma_gather call
    SCW: int = 4         # windows per h-store dma
    OCW: int = 4         # windows per final output dma
    G: int = 1           # AllGather chunks (window groups)
    DVE_EVERY: int = 0   # 1 of every DVE_EVERY sel-tiles built on DVE (0: none)
    ACT_EVERY: int = 0   # 1 of every ACT_EVERY sel-tiles built on ScalarE (0: none)

    @property
    def NSH(self):
        return self.N // self.C

    @property
    def BS(self):
        return self.N // self.NB

    @property
    def SLOTS(self):
        return self.WN * 128

    @property
    def CALLCH(self):
        return self.CALL // 128

    @property
    def WCH(self):
        # window group boundaries for AllGather chunks
        per = -(-self.WN // self.G)
        return [min(self.WN, per * g) for g in range(self.G + 1)]


CFG = Cfg()


# ---------------------------------------------------------------- host prep


def _balance_core(dnb, WN, cap=128, ctarget=512):
    """Assign nodes (rows of dnb, per-block in-degree vectors) to WN bins of
    <=cap nodes, aiming for per-(bin, block) sums <= target. Overflow (when a
    block's total exceeds WN*ctarget) is concentrated in the LAST windows.
    Returns (bin id per node, binsum)."""
    nn, NB = dnb.shape
    T = dnb.sum(0)
    # per-block overflow chunks, assigned to tail windows
    target = np.full((WN, NB), ctarget, np.int64)
    for b in range(NB):
        q = max(0, -(-int(T[b] - WN * ctarget) // 128))
        for i in range(min(q, WN)):
            target[WN - 1 - i, b] += 128
    tot = dnb.sum(1)
    order = np.argsort(-tot, kind="stable")
    binsum = np.zeros((WN, NB), np.int64)
    binslots = np.zeros(WN, np.int64)
    assign = np.full(nn, -1, np.int64)
    tgt = target.astype(np.float64)
    for n in order:
        dv = dnb[n]
        fill = ((binsum + dv) / tgt).max(axis=1)
        fill += 1e-5 * binslots
        fill[binslots >= cap] = 1e30
        j = int(np.argmin(fill))
        assign[n] = j
        binsum[j] += dv
        binslots[j] += 1

    # repair: evict small-degree nodes from violated cells into bins with
    # slack (move if a slot is free, else swap with a light partner)
    for _ in range(30):
        viol = np.argwhere(binsum > target)
        if len(viol) == 0:
            break
        moved = 0
        for j, b in viol:
            guard = 0
            while binsum[j, b] > target[j, b] and guard < 64:
                guard += 1
                members = np.where(assign == j)[0]
                mb = dnb[members, b]
                cand_n = members[mb > 0]
                if len(cand_n) == 0:
                    break
                # smallest positive contribution first
                cand_n = cand_n[np.argsort(dnb[cand_n, b], kind="stable")]
                done = False
                for n in cand_n[:8]:
                    dv = dnb[n]
                    ok = ((binsum + dv) <= target).all(axis=1) & (binslots < cap)
                    ok[j] = False
                    cand = np.where(ok)[0]
                    if len(cand):
                        j2 = int(cand[np.argmin(((binsum[cand] + dv) / target[cand]).max(1))])
                        assign[n] = j2
                        binsum[j] -= dv
                        binsum[j2] += dv
                        binslots[j] -= 1
                        binslots[j2] += 1
                        moved += 1
                        done = True
                        break
                    # swap with the lightest partner in low-fill bins
                    for j2 in np.argsort(binsum[:, b])[:24]:
                        if j2 == j:
                            continue
                        mem2 = np.where(assign == j2)[0]
                        if len(mem2) == 0:
                            continue
                        m = mem2[np.argmin(dnb[mem2, b])]
                        dm = dnb[m]
                        if dm[b] >= dv[b]:
                            continue
                        nj = binsum[j] - dv + dm
                        nj2 = binsum[j2] - dm + dv
                        if (nj <= target[j]).all() and (nj2 <= target[j2]).all():
                            assign[n], assign[m] = j2, j
                            binsum[j] = nj
                            binsum[j2] = nj2
                            moved += 1
                            done = True
                            break
                    if done:
                        break
                if not done:
                    break
        if moved == 0:
            break
    return assign, binsum


def _layer_sched(counts, cfg):
    """counts: [C, WN, NBL] per-core cell sizes -> shared schedule dict."""
    K = np.ceil(counts / 128).astype(np.int64).max(axis=0)  # [WN, NBL]
    TCH = int(K.sum())
    Sb = (K.sum(axis=0) * 128).astype(np.int64)             # [NBL] idx slots
    ncalls = np.ceil(Sb / cfg.CALL).astype(np.int64)
    lastvalid = Sb - (ncalls - 1) * cfg.CALL
    # DVE / streamed-sval split, round-robin by chunk column
    if cfg.DVE_EVERY > 0:
        is_dve = (np.arange(TCH) % cfg.DVE_EVERY) == 0
    else:
        is_dve = np.zeros(TCH, bool)
    if cfg.ACT_EVERY > 0:
        is_act = (~is_dve) & ((np.arange(TCH) % cfg.ACT_EVERY) == 1)
    else:
        is_act = np.zeros(TCH, bool)
    is_dve = is_dve | is_act                 # "on-chip" tiles (drel-driven)
    dcol_of = np.cumsum(is_dve) - 1          # on-chip drel column index
    svi_of = np.cumsum(~is_dve) - 1          # streamed sval tile index
    NDV = int(is_dve.sum())
    NSV = TCH - NDV
    NBL = K.shape[1]
    GCOLS_B = [int(ncalls[b]) * (cfg.CALL // 16) for b in range(NBL)]
    GOFF = np.concatenate([[0], np.cumsum(GCOLS_B)]).astype(int)
    return dict(K=K, TCH=TCH, Sb=Sb, ncalls=ncalls, lastvalid=lastvalid,
                is_dve=is_dve, is_act=is_act, dcol_of=dcol_of, svi_of=svi_of,
                NDV=NDV, NSV=max(NSV, 1), GOFF=GOFF)


def _core_layer_maps(sch, cellcnt, rel_src, ep, ev, cfg):
    """Build gather idx stream + per-chunk metadata for one (core, layer).
    rel_src: int64 block-relative row index per edge (cell-sorted order).
    Returns dict with gidx [128, GCOLS], drel/vval [128, NDV] f32,
    sval [128, NSV*128] bf16."""
    WN, CALL = cfg.WN, cfg.CALL
    K, TCH, Sb, ncalls = sch["K"], sch["TCH"], sch["Sb"], sch["ncalls"]
    is_dve, dcol_of, svi_of = sch["is_dve"], sch["dcol_of"], sch["svi_of"]
    NDV, NSV = sch["NDV"], sch["NSV"]
    NBL = K.shape[1]

    gstreams = [np.zeros(int(ncalls[b]) * CALL, np.int16) for b in range(NBL)]
    for b in range(NBL):
        if Sb[b] < ncalls[b] * CALL:
            gstreams[b][Sb[b]:] = -1  # tail of last call: skipped by DMA
    drel = np.full((max(NDV, 1), 128), -255.0, np.float32)
    vval = np.zeros((max(NDV, 1), 128), np.float32)
    sval = np.zeros((NSV, 128, 128), f8np)

    eoff = 0
    gcol = 0
    posb = np.zeros(NBL, np.int64)
    for w in range(WN):
        for b in range(NBL):
            L = int(cellcnt[w, b])
            kwb = int(K[w, b])
            if kwb == 0:
                assert L == 0
                continue
            sl = slice(eoff, eoff + L)
            st = int(posb[b])
            gstreams[b][st:st + L] = rel_src[sl].astype(np.int16)
            epc = ep[sl]
            evc = ev[sl]
            for kk in range(kwb):
                r0, r1 = kk * 128, min((kk + 1) * 128, L)
                gc = gcol + kk
                if is_dve[gc]:
                    dc = int(dcol_of[gc])
                    if r1 > r0:
                        drel[dc, :r1 - r0] = -epc[r0:r1]
                        vval[dc, :r1 - r0] = evc[r0:r1].astype(np.float32)
                else:
                    si = int(svi_of[gc])
                    if r1 > r0:
                        sval[si, np.arange(r1 - r0), epc[r0:r1]] = 1.0
            posb[b] += kwb * 128
            gcol += kwb
            eoff += L
    assert gcol == TCH

    gparts = []
    for b in range(NBL):
        arr = gstreams[b].reshape(-1, 16).T  # [16, Sb_pad/16]
        gparts.append(arr)
    gidx16 = np.concatenate(gparts, axis=1)          # [16, GCOLS]
    gidx = np.tile(gidx16, (8, 1)).astype(np.int16)  # [128, GCOLS]
    return dict(
        gidx=gidx,
        drel=np.ascontiguousarray(drel.T), vval=np.ascontiguousarray(vval.T),
        sval=np.ascontiguousarray(sval.transpose(1, 0, 2).reshape(128, -1)),
    ), eoff


def prep(x, edge_index, cfg=CFG):
    """Host-side sharding/schedule. Returns (schedule, per-core input maps)."""
    C, NB, WN, NSH, BS = cfg.C, cfg.NB, cfg.WN, cfg.NSH, cfg.BS
    SLOTS, G = cfg.SLOTS, cfg.G
    WCH = cfg.WCH
    src = np.asarray(edge_index[0]).astype(np.int64)
    dst = np.asarray(edge_index[1]).astype(np.int64)

    deg = np.bincount(dst, minlength=cfg.N).astype(np.float64)
    invdeg = (1.0 / np.maximum(deg, 1.0)).astype(np.float32)
    vedge_all = invdeg[dst].astype(bfnp)

    ecore = dst // NSH
    eblock1 = src // BS

    # --- per-core balance: node-local id -> (window, pos)
    win_of = np.zeros(cfg.N, np.int64)   # window within core
    pos_of = np.zeros(cfg.N, np.int64)   # slot within window
    counts1 = np.zeros((C, WN, NB), np.int64)
    for c in range(C):
        lo = c * NSH
        dnb = np.zeros((NSH, NB), np.int64)
        emask = ecore == c
        np.add.at(dnb, (dst[emask] - lo, eblock1[emask]), 1)
        assign, binsum = _balance_core(dnb, WN)
        # order bins by descending per-block chunk tuple so heavy cells align
        # at the same window index across cores
        kt = np.ceil(binsum / 128).astype(np.int64)
        key = [tuple(-kt[j]) + tuple(-binsum[j]) for j in range(WN)]
        order = sorted(range(WN), key=lambda j: key[j])
        rank = np.empty(WN, np.int64)
        rank[order] = np.arange(WN)
        w = rank[assign]
        win_of[lo:lo + NSH] = w
        # position within window: stable by node id
        order2 = np.lexsort((np.arange(NSH), w))
        pos = np.zeros(NSH, np.int64)
        pcount = np.zeros(WN, np.int64)
        for m in order2:
            pos[m] = pcount[w[m]]
            pcount[w[m]] += 1
        pos_of[lo:lo + NSH] = pos
        cnt = np.zeros((WN, NB), np.int64)
        np.add.at(cnt, (w[dst[emask] - lo], eblock1[emask]), 1)
        counts1[c] = cnt

    # layer-2 geometry: hfull is chunk-major: for AllGather chunk g
    # (windows [WCH[g], WCH[g+1])), node row = cbase[g] + core*crows[g] +
    # (win - WCH[g])*128 + pos; gather block g covers rows
    # [cbase[g], cbase[g+1]).
    if G == 1:
        # core-major hfull: blocks of 2 cores (2*NSH == BS) share layer-1's
        # cell structure exactly
        g2_of = (np.arange(cfg.N) // NSH) * SLOTS + win_of * 128 + pos_of
        eblock2 = eblock1
        cbase = np.arange(NB + 1, dtype=np.int64) * 2 * SLOTS
        counts2 = counts1
    else:
        crows = np.array([(WCH[g + 1] - WCH[g]) * 128 for g in range(G)], np.int64)
        cbase = np.concatenate([[0], np.cumsum(crows * C)]).astype(np.int64)
        for g in range(G):
            assert crows[g] * C <= 32767, (g, crows[g] * C)
        gchunk_of_w = np.zeros(WN, np.int64)
        for g in range(G):
            gchunk_of_w[WCH[g]:WCH[g + 1]] = g
        wch_arr = np.asarray(WCH)
        vg = gchunk_of_w[win_of]
        g2_of = (cbase[vg] + (np.arange(cfg.N) // NSH) * crows[vg]
                 + (win_of - wch_arr[vg]) * 128 + pos_of)
        eblock2 = vg[src]
        counts2 = np.zeros((C, WN, G), np.int64)
        for c in range(C):
            emask = ecore == c
            np.add.at(counts2[c], (win_of[dst[emask]], eblock2[emask]), 1)

    sch1 = _layer_sched(counts1, cfg)
    sch2 = _layer_sched(counts2, cfg)

    ewin = win_of[dst]
    epos = pos_of[dst]

    in_maps = []
    for c in range(C):
        lo = c * NSH
        emask = ecore == c
        es = src[emask]
        ew, eb1, eb2 = ewin[emask], eblock1[emask], eblock2[emask]
        ep = epos[emask]
        ev = vedge_all[emask]
        eg2 = g2_of[es]

        # layer 1: cells (w, src//BS), sorted by src within cell
        o1 = np.lexsort((es, eb1, ew))
        m1, ne1 = _core_layer_maps(
            sch1, counts1[c], (es - eb1 * BS)[o1], ep[o1], ev[o1], cfg)
        assert ne1 == es.shape[0]
        # layer 2: cells (w, chunk(src)), sorted by hfull row within cell
        rel2 = eg2 - cbase[eb2]
        o2 = np.lexsort((rel2, eb2, ew))
        m2, ne2 = _core_layer_maps(
            sch2, counts2[c], rel2[o2], ep[o2], ev[o2], cfg)
        assert ne2 == es.shape[0]

        # slot s = win*128+pos of node-local rows; dummy slots unused
        msk = np.arange(cfg.N)[lo:lo + NSH]
        sl_idx = win_of[msk] * 128 + pos_of[msk]
        sl_nodes = np.full(cfg.SLOTS, -1, np.int64)
        sl_nodes[sl_idx] = np.arange(NSH)

        # xT in slot order
        xT = np.zeros((cfg.D, cfg.SLOTS), bfnp)
        xT[:, sl_idx] = np.asarray(x[lo:lo + NSH]).astype(bfnp).T
        invsl = np.ones((cfg.SLOTS,), np.float32)
        invsl[sl_idx] = invdeg[lo:lo + NSH]
        bcast = np.ascontiguousarray(
            np.broadcast_to(invsl.astype(bfnp), (128, cfg.SLOTS)))

        in_maps.append(dict(
            gidx1=m1["gidx"], dstrel1=m1["drel"], vval1=m1["vval"], sval1=m1["sval"],
            gidx2=m2["gidx"], dstrel2=m2["drel"], vval2=m2["vval"], sval2=m2["sval"],
            xT=np.ascontiguousarray(xT), bcast=bcast,
            slot_nodes=sl_nodes,                   # host-only
        ))

    sched = dict(sch1=sch1, sch2=sch2, cbase=cbase,
                 TCH=sch1["TCH"] + sch2["TCH"])
    return sched, in_maps


# ---------------------------------------------------------------- program


def build(cfg, sched):
    sch = {1: sched["sch1"], 2: sched["sch2"]}
    cbase = sched["cbase"]
    C, D, NB, WN, BS = cfg.C, cfg.D, cfg.NB, cfg.WN, cfg.BS
    CALL, CALLCH, SLOTS, G = cfg.CALL, cfg.CALLCH, cfg.SLOTS, cfg.G
    WCH = cfg.WCH

    nc = bacc.Bacc(None, num_devices=C, num_swdge_queues=4,
                   dynamic_dma_scratch_size=24576)
    x_d = nc.dram_tensor("xbf", [cfg.N, D], BF, kind="ExternalInput")
    xT_d = nc.dram_tensor("xT", [D, SLOTS], BF, kind="ExternalInput")
    gidx_d, drel_d, vval_d, sval_d = {}, {}, {}, {}
    for L in (1, 2):
        s = sch[L]
        gidx_d[L] = nc.dram_tensor(f"gidx{L}", [128, int(s["GOFF"][-1])], I16,
                                   kind="ExternalInput")
        drel_d[L] = nc.dram_tensor(f"dstrel{L}", [128, max(s["NDV"], 1)], F32,
                                   kind="ExternalInput")
        vval_d[L] = nc.dram_tensor(f"vval{L}", [128, max(s["NDV"], 1)], F32,
                                   kind="ExternalInput")
        sval_d[L] = nc.dram_tensor(f"sval{L}", [128, s["NSV"] * 128], FP8,
                                   kind="ExternalInput")
    w_d = {}
    for nm in ("wlt1", "wrt1", "wlt2", "wrt2"):
        w_d[nm] = nc.dram_tensor(nm, [D, D], BF, kind="ExternalInput")
    b1_d = nc.dram_tensor("b1c", [D, 1], F32, kind="ExternalInput")
    b2_d = nc.dram_tensor("b2r", [1, D], F32, kind="ExternalInput")
    bc_d = nc.dram_tensor("bcast", [128, SLOTS], BF, kind="ExternalInput")
    out_d = nc.dram_tensor("out", [SLOTS, D], BF, kind="ExternalOutput")

    ident_d = nc.inline_tensor(np.eye(128, dtype=bfnp), "identc")
    iota_d = nc.inline_tensor(
        np.broadcast_to(np.arange(128, dtype=bfnp), (128, 128)).copy(), "iotac")
    ones_d = nc.inline_tensor(np.ones((1, 128), np.float32), "onesc")

    hsl_d = nc.dram_tensor("hslots", [SLOTS, D], BF)  # Internal, slot order
    hfull_d = nc.dram_tensor("hfull", [C * SLOTS, D], BF, addr_space="Shared")

    with tile.TileContext(nc) as tc, ExitStack() as ctx:
        const = ctx.enter_context(tc.tile_pool(name="const", bufs=1))
        meta = ctx.enter_context(tc.tile_pool(name="meta", bufs=1))
        gpool = ctx.enter_context(tc.tile_pool(name="gather", bufs=5))
        spool = ctx.enter_context(tc.tile_pool(name="sv", bufs=6))
        ohp = ctx.enter_context(tc.tile_pool(name="oh", bufs=1))
        dhp = ctx.enter_context(tc.tile_pool(name="dh", bufs=1))
        mwp = ctx.enter_context(tc.tile_pool(name="mw", bufs=6))
        htp = ctx.enter_context(tc.tile_pool(name="ht", bufs=1))
        xtp = ctx.enter_context(tc.tile_pool(name="xt", bufs=2))
        stgp = ctx.enter_context(tc.tile_pool(name="stg", bufs=2))
        ostgp = ctx.enter_context(tc.tile_pool(name="ostg", bufs=2))
        psA = ctx.enter_context(tc.tile_pool(name="psA", bufs=4, space="PSUM"))
        psB = ctx.enter_context(tc.tile_pool(name="psB", bufs=2, space="PSUM"))
        psT = ctx.enter_context(tc.tile_pool(name="psT", bufs=2, space="PSUM"))
        bcp = ctx.enter_context(tc.tile_pool(name="bc", bufs=1))

        def load(pool, dram, shape, dtype):
            t = pool.tile(shape, dtype, tag=dram.name)
            nc.sync.dma_start(t[:], dram[:])
            return t

        ident_s = load(const, ident_d, [128, 128], BF)
        iota_s = load(const, iota_d, [128, 128], BF)
        ones_s = load(const, ones_d, [1, 128], F32)
        w_s = {nm: load(const, w_d[nm], [D, D], BF) for nm in w_d}
        b1_s = load(const, b1_d, [D, 1], F32)
        b2_s = load(const, b2_d, [1, D], F32)

        drel_s = {L: load(meta, drel_d[L], [128, max(sch[L]["NDV"], 1)], F32)
                  for L in (1, 2)}
        vval_s = {L: load(meta, vval_d[L], [128, max(sch[L]["NDV"], 1)], F32)
                  for L in (1, 2)}
        gidx_s = {}
        for L in (1, 2):
            gidx_s[L] = meta.tile([128, int(sch[L]["GOFF"][-1])], I16,
                                  tag=f"gidx{L}", name=f"gidx{L}")

        def load_gidx(L):
            GOFF = sch[L]["GOFF"]
            for _b in range(sch[L]["K"].shape[1]):
                nc.sync.dma_start(gidx_s[L][:, int(GOFF[_b]):int(GOFF[_b + 1])],
                                  gidx_d[L][:, int(GOFF[_b]):int(GOFF[_b + 1])])

        load_gidx(1)
        # prefetch the first sval groups of layer 1 ahead of the bulk loads
        prefetched = {}
        for _si in range(2):
            _nch = min(8, sch[1]["NSV"] - _si * 8)
            _stt = spool.tile([128, 8, 128], FP8, tag="sv", name=f"sv1_{_si}")
            nc.sync.dma_start(_stt[:, :_nch, :],
                              sval_d[1][:, _si * 1024:_si * 1024 + _nch * 128])
            prefetched[_si] = _stt
        bc_s = load(bcp, bc_d, [128, SLOTS], BF)
        hT_s = htp.tile([D, SLOTS], BF, tag="hT")

        def run_layer(L, w_lo, w_hi):
            s = sch[L]
            K, ncalls, lastvalid = s["K"], s["ncalls"], s["lastvalid"]
            is_dve, dcol_of, svi_of, NSV = (s["is_dve"], s["dcol_of"],
                                            s["svi_of"], s["NSV"])
            GOFF = s["GOFF"]
            NBL = K.shape[1]
            src_d = x_d if L == 1 else hfull_d
            drl, vvl, gix = drel_s[L], vval_s[L], gidx_s[L]
            st_ = run_layer.state.setdefault(
                L, dict(posb=[0] * NBL, gt=[None] * NBL, gcol=0, stile=None,
                        stg=None, ostg=None, bct=None, xtt=None))
            posb, gt = st_["posb"], st_["gt"]
            SCW, OCW = cfg.SCW, cfg.OCW
            for w in range(w_lo, w_hi):
                nchunks_w = int(K[w].sum())
                psum_a = psA.tile([128, 128], F32, tag="agg", name=f"agg{L}_{w}")
                ci = 0
                for b in range(NBL):
                    for k in range(int(K[w, b])):
                        pos = posb[b]
                        call_i, col = divmod(pos, CALLCH)
                        if col == 0:
                            gt[b] = gpool.tile([128, CALLCH, 128], BF, tag=f"g{b}",
                                               name=f"g{b}_{L}_{call_i}")
                            nvalid = CALL if call_i < int(ncalls[b]) - 1 else int(lastvalid[b])
                            ioff = GOFF[b] + call_i * (CALL // 16)
                            if L == 1:
                                in_ap = src_d[b * BS:(b + 1) * BS, :]
                            else:
                                in_ap = src_d[int(cbase[b]):int(cbase[b + 1]), :]
                            nc.gpsimd.dma_gather(
                                out_ap=gt[b][:],
                                in_ap=in_ap,
                                idxs_ap=gix[:, ioff:ioff + CALL // 16],
                                num_idxs=CALL,
                                num_idxs_reg=nvalid,
                                elem_size=D,
                            )
                        gcol = st_["gcol"]
                        if is_dve[gcol]:
                            dc = int(dcol_of[gcol])
                            dd = dhp.tile([128, 128], BF, tag="dh",
                                          name=f"dh{L}_{gcol}")
                            nc.scalar.activation(
                                dd[:], iota_s[:],
                                mybir.ActivationFunctionType.Abs,
                                bias=drl[:, dc:dc + 1], scale=1.0)
                            S = ohp.tile([128, 128], BF, tag="oh",
                                         name=f"oh{L}_{gcol}")
                            nc.scalar.activation(
                                S[:], dd[:],
                                mybir.ActivationFunctionType.Relu,
                                bias=1.0, scale=-1.0)
                            rhs_ap = S[:]
                        else:
                            si, sc = divmod(int(svi_of[gcol]), 8)
                            if sc == 0:
                                if L == 1 and si in prefetched:
                                    st_["stile"] = prefetched.pop(si)
                                else:
                                    nch = min(8, NSV - si * 8)
                                    stt = spool.tile([128, 8, 128], FP8, tag="sv",
                                                     name=f"sv{L}_{si}")
                                    nc.sync.dma_start(
                                        stt[:, :nch, :],
                                        sval_d[L][:, si * 1024:si * 1024 + nch * 128])
                                    st_["stile"] = stt
                            rhs_ap = st_["stile"][:, sc, :]
                        nc.tensor.matmul(
                            out=psum_a[:], lhsT=gt[b][:, col, :], rhs=rhs_ap,
                            start=(ci == 0), stop=(ci == nchunks_w - 1),
                        )
                        st_["gcol"] += 1
                        posb[b] += 1
                        ci += 1
                m_s = mwp.tile([128, 128], BF, tag="mw", name=f"mw{L}_{w}")
                wsl = slice(w * 128, (w + 1) * 128)
                if nchunks_w:
                    nc.vector.tensor_tensor(out=m_s[:], in0=psum_a[:],
                                            in1=bc_s[:, wsl],
                                            op=mybir.AluOpType.mult)
                else:
                    nc.vector.memset(m_s[:], 0.0)
                if L == 1:
                    psum_h = psB.tile([128, 128], F32, tag="h", name=f"h{L}_{w}")
                    nc.tensor.matmul(out=psum_h[:], lhsT=w_s["wlt1"][:], rhs=m_s[:],
                                     start=True, stop=False)
                    xgi, xgo = divmod(w, 8)
                    if xgo == 0 or st_["xtt"] is None:
                        nxw = min(8, WN - xgi * 8)
                        xtt = xtp.tile([128, 8, 128], BF, tag="xT",
                                       name=f"xT{xgi}")
                        nc.sync.dma_start(
                            xtt[:, :nxw, :],
                            xT_d[:, xgi * 1024:xgi * 1024 + nxw * 128])
                        st_["xtt"] = xtt
                    nc.tensor.matmul(out=psum_h[:], lhsT=w_s["wrt1"][:],
                                     rhs=st_["xtt"][:, xgo, :], start=False,
                                     stop=True)
                    nc.scalar.activation(hT_s[:, wsl], psum_h[:],
                                         mybir.ActivationFunctionType.Identity,
                                         bias=b1_s[:, 0:1], scale=1.0)
                    psum_t = psT.tile([128, 128], BF, tag="tr", name=f"tr{w}")
                    nc.tensor.transpose(psum_t[:], hT_s[:, wsl], ident_s[:])
                    wi = (w - w_lo) % SCW
                    if wi == 0:
                        st_["stg"] = stgp.tile([128, SCW, 128], BF, tag="stg",
                                               name=f"stg{w}")
                    nc.scalar.copy(st_["stg"][:, wi, :], psum_t[:])
                    if wi == SCW - 1 or w == w_hi - 1:
                        used = wi + 1
                        w0 = w - wi
                        hap = hsl_d[:].rearrange("(w p) f -> p w f", p=128)
                        nc.sync.dma_start(hap[:, w0:w0 + used, :],
                                          st_["stg"][:, :used, :])
                else:
                    psum_h = psB.tile([128, 128], F32, tag="h", name=f"h{L}_{w}")
                    nc.tensor.matmul(out=psum_h[:], lhsT=m_s[:], rhs=w_s["wlt2"][:],
                                     start=True, stop=False)
                    nc.tensor.matmul(out=psum_h[:], lhsT=hT_s[:, wsl],
                                     rhs=w_s["wrt2"][:], start=False, stop=False)
                    nc.tensor.matmul(out=psum_h[:], lhsT=ones_s[0:1, :],
                                     rhs=b2_s[0:1, :], start=False, stop=True)
                    wi = w % OCW
                    if wi == 0:
                        st_["ostg"] = ostgp.tile([128, OCW, 128], BF, tag="ostg",
                                                 name=f"ostg{w}")
                    nc.scalar.copy(st_["ostg"][:, wi, :], psum_h[:])
                    if wi == OCW - 1 or w == WN - 1:
                        used = wi + 1
                        w0 = w - wi
                        oap = out_d[:].rearrange("(w p) f -> p w f", p=128)
                        nc.sync.dma_start(oap[:, w0:w0 + used, :],
                                          st_["ostg"][:, :used, :])

        run_layer.state = {}
        # layer 1 in window chunks; AllGather each chunk as soon as its
        # windows are stored (overlaps the collective with remaining compute)
        for g in range(G):
            run_layer(1, WCH[g], WCH[g + 1])
            if g == G - 1:
                load_gidx(2)
            r0, r1 = WCH[g] * 128, WCH[g + 1] * 128
            out_lo = 0 if G == 1 else int(cbase[g])
            out_hi = C * SLOTS if G == 1 else int(cbase[g + 1])
            nc.gpsimd.collective_compute(
                "AllGather", mybir.AluOpType.bypass,
                replica_groups=[list(range(C))],
                ins=[hsl_d[r0:r1, :]],
                outs=[hfull_d[out_lo:out_hi, :]],
            )
        run_layer(2, 0, WN)

    # spread SWDGE gather descriptor generation across the 4 SWDGE queues
    # (parallel Q7 pairs). Tile assigned DMASW lanes round-robin in scheduled
    # order; keep sem-lane <-> queue binding consistent by deriving the queue
    # from the lane (lane % 4).
    from concourse.tile_sem_assignment import PROC_NAME_TO_IDX
    dmasw0 = PROC_NAME_TO_IDX["DMASW0"]
    for inst in nc.inst_map.values():
        if isinstance(inst, (mybir.InstDMAGatherAnt, mybir.InstDMAScatterAddAnt)):
            proc = getattr(inst, "bass_scheduled_proc", None)
            if proc is not None and dmasw0 <= proc < dmasw0 + 8:
                inst.queue_num = (proc - dmasw0) % 4

    nc.compile()
    return nc


# ---------------------------------------------------------------- kernel


def kernel(**inputs):
    cfg = CFG
    x = np.asarray(inputs["x"], np.float32)
    ei = np.asarray(inputs["edge_index"])
    sched, in_maps = prep(x, ei, cfg)
    nc = build(cfg, sched)

    x_bf = x.astype(bfnp)
    shared = dict(
        xbf=x_bf,
        wlt1=np.ascontiguousarray(np.asarray(inputs["Wl1"], np.float32).T.astype(bfnp)),
        wrt1=np.ascontiguousarray(np.asarray(inputs["Wr1"], np.float32).T.astype(bfnp)),
        wlt2=np.ascontiguousarray(np.asarray(inputs["Wl2"], np.float32).T.astype(bfnp)),
        wrt2=np.ascontiguousarray(np.asarray(inputs["Wr2"], np.float32).T.astype(bfnp)),
        b1c=np.asarray(inputs["b1"], np.float32).reshape(cfg.D, 1).copy(),
        b2r=np.asarray(inputs["b2"], np.float32).reshape(1, cfg.D).copy(),
    )
    slot_nodes = [m.pop("slot_nodes") for m in in_maps]
    run_maps = [dict(shared, **{k: v for k, v in m.items()}) for m in in_maps]

    res = None
    last_err = None
    for attempt in range(3):
        try:
            res = run_bass_kernel_spmd(nc, run_maps, core_ids=list(range(cfg.C)))
            break
        except Exception as e:  # transient device wedge: retry
            last_err = e
            import time
            time.sleep(10)
    if res is None:
        raise last_err
    out = np.empty((cfg.N, cfg.D), np.float32)
    for c in range(cfg.C):
        oc = res.results[c]["out"]
        sn = slot_nodes[c]
        real = sn >= 0
        out[c * cfg.NSH + sn[real]] = oc[real]
    return out


if __name__ == "__main__":
    d = np.load("/tmp/inputs.npz")
    ins = {k: d[k] for k in ("x", "edge_index", "Wl1", "Wr1", "b1", "Wl2", "Wr2", "b2")}
    got = kernel(**ins)
    exp = d["expected"]
    err = np.abs(got - exp).max() / np.abs(exp).max()
    print("Relative error:", err)


# revision 39
# speedup vs baseline: 1.0688x; 1.0084x over previous
"""GraphSAGE 2-layer (SAGEConv mean-aggregation) Bass kernel for 8 TRN2 NeuronCores.

Strategy (see spec sharding_hint):
  - Destination nodes sharded across 8 cores (12500/core). Within each core a
    greedy balancer assigns nodes to 98 windows x 128 slots so that each
    (window, src-block) cell has <= ~512 edges -> near-uniform SPMD schedule.
  - Edges partitioned by destination core, sorted by (window, src-block) and
    by source row within each cell (HBM locality for the gathers).
  - Aggregation: dma_gather pulls rows (bf16) from HBM in 4 source blocks
    (int16 index limit); selection tiles (pure 0/1 one-hot, [128 edges x 128
    slots]) are streamed pre-built from HBM in fp8 (half the bytes of bf16,
    exactly representable); TensorE accumulates raw neighbor sums^T per
    window in PSUM. The mean division (invdeg per destination slot) is
    applied by one DVE tensor_tensor(mult) per window against an SBUF-
    resident [128, SLOTS] broadcast tile of per-slot inverse degrees.
  - Transform per window: two 128x128 matmuls (+ bias) produce hT; layer-1 h
    is transposed to row-major SLOT order and written with plain DMA (no
    scatter). One AllGather of the slot-ordered shard forms hfull; layer-2
    gathers reference (core, slot) coordinates directly - 2*NSH == BS so
    layer-2 blocks coincide with layer-1 source blocks and the cell
    structure is shared.
  - Final layer-2 output is written in slot order (bf16) and inverse-
    permuted on host.
"""

import sys

sys.path.insert(0, "/opt/trn_rl_repo")

from contextlib import ExitStack
from dataclasses import dataclass

import ml_dtypes
import numpy as np

import concourse.bacc as bacc
import concourse.bass as bass
import concourse.mybir as mybir
import concourse.tile as tile
from concourse.bass_utils import run_bass_kernel_spmd

BF = mybir.dt.bfloat16
F32 = mybir.dt.float32
I16 = mybir.dt.int16
bfnp = ml_dtypes.bfloat16
f8np = ml_dtypes.float8_e4m3
FP8 = mybir.dt.float8e4


@dataclass
class Cfg:
    N: int = 100000      # total nodes
    D: int = 128         # feature dim
    C: int = 8           # cores
    NB: int = 4          # source blocks (int16 gather index limit)
    WN: int = 98         # windows per core (128 dst nodes each)
    CALL: int = 1024     # gather indices per dma_gather call
    SCW: int = 4         # windows per h-store dma
    OCW: int = 4         # windows per final output dma
    G: int = 1           # AllGather chunks (window groups)
    DVE_EVERY: int = 0   # 1 of every DVE_EVERY sel-tiles built on DVE (0: none)
    ACT_EVERY: int = 0   # 1 of every ACT_EVERY sel-tiles built on ScalarE (0: none)

    @property
    def NSH(self):
        return self.N // self.C

    @property
    def BS(self):
        return self.N // self.NB

    @property
    def SLOTS(self):
        return self.WN * 128

    @property
    def CALLCH(self):
        return self.CALL // 128

    @property
    def WCH(self):
        # window group boundaries for AllGather chunks
        per = -(-self.WN // self.G)
        return [min(self.WN, per * g) for g in range(self.G + 1)]


CFG = Cfg()


# ---------------------------------------------------------------- host prep


def _balance_core(dnb, WN, cap=128, ctarget=512):
    """Assign nodes (rows of dnb, per-block in-degree vectors) to WN bins of
    <=cap nodes, aiming for per-(bin, block) sums <= target. Overflow (when a
    block's total exceeds WN*ctarget) is concentrated in the LAST windows.
    Returns (bin id per node, binsum)."""
    nn, NB = dnb.shape
    T = dnb.sum(0)
    # per-block overflow chunks, assigned to tail windows
    target = np.full((WN, NB), ctarget, np.int64)
    for b in range(NB):
        q = max(0, -(-int(T[b] - WN * ctarget) // 128))
        for i in range(min(q, WN)):
            target[WN - 1 - i, b] += 128
    tot = dnb.sum(1)
    order = np.argsort(-tot, kind="stable")
    binsum = np.zeros((WN, NB), np.int64)
    binslots = np.zeros(WN, np.int64)
    assign = np.full(nn, -1, np.int64)
    tgt = target.astype(np.float64)
    for n in order:
        dv = dnb[n]
        fill = ((binsum + dv) / tgt).max(axis=1)
        fill += 1e-5 * binslots
        fill[binslots >= cap] = 1e30
        j = int(np.argmin(fill))
        assign[n] = j
        binsum[j] += dv
        binslots[j] += 1

    # repair: evict small-degree nodes from violated cells into bins with
    # slack (move if a slot is free, else swap with a light partner)
    for _ in range(30):
        viol = np.argwhere(binsum > target)
        if len(viol) == 0:
            break
        moved = 0
        for j, b in viol:
            guard = 0
            while binsum[j, b] > target[j, b] and guard < 64:
                guard += 1
                members = np.where(assign == j)[0]
                mb = dnb[members, b]
                cand_n = members[mb > 0]
                if len(cand_n) == 0:
                    break
                # smallest positive contribution first
                cand_n = cand_n[np.argsort(dnb[cand_n, b], kind="stable")]
                done = False
                for n in cand_n[:8]:
                    dv = dnb[n]
                    ok = ((binsum + dv) <= target).all(axis=1) & (binslots < cap)
                    ok[j] = False
                    cand = np.where(ok)[0]
                    if len(cand):
                        j2 = int(cand[np.argmin(((binsum[cand] + dv) / target[cand]).max(1))])
                        assign[n] = j2
                        binsum[j] -= dv
                        binsum[j2] += dv
                        binslots[j] -= 1
                        binslots[j2] += 1
                        moved += 1
                        done = True
                        break
                    # swap with the lightest partner in low-fill bins
                    for j2 in np.argsort(binsum[:, b])[:24]:
                        if j2 == j:
                            continue
                        mem2 = np.where(assign == j2)[0]
                        if len(mem2) == 0:
                            continue
                        m = mem2[np.argmin(dnb[mem2, b])]
                        dm = dnb[m]
                        if dm[b] >= dv[b]:
                            continue
                        nj = binsum[j] - dv + dm
                        nj2 = binsum[j2] - dm + dv
                        if (nj <= target[j]).all() and (nj2 <= target[j2]).all():
                            assign[n], assign[m] = j2, j
                            binsum[j] = nj
                            binsum[j2] = nj2
                            moved += 1
                            done = True
                            break
                    if done:
                        break
                if not done:
                    break
        if moved == 0:
            break
    return assign, binsum


def _layer_sched(counts, cfg):
    """counts: [C, WN, NBL] per-core cell sizes -> shared schedule dict."""
    K = np.ceil(counts / 128).astype(np.int64).max(axis=0)  # [WN, NBL]
    TCH = int(K.sum())
    Sb = (K.sum(axis=0) * 128).astype(np.int64)             # [NBL] idx slots
    ncalls = np.ceil(Sb / cfg.CALL).astype(np.int64)
    lastvalid = Sb - (ncalls - 1) * cfg.CALL
    # DVE / streamed-sval split, round-robin by chunk column
    if cfg.DVE_EVERY > 0:
        is_dve = (np.arange(TCH) % cfg.DVE_EVERY) == 0
    else:
        is_dve = np.zeros(TCH, bool)
    if cfg.ACT_EVERY > 0:
        is_act = (~is_dve) & ((np.arange(TCH) % cfg.ACT_EVERY) == 1)
    else:
        is_act = np.zeros(TCH, bool)
    is_dve = is_dve | is_act                 # "on-chip" tiles (drel-driven)
    dcol_of = np.cumsum(is_dve) - 1          # on-chip drel column index
    svi_of = np.cumsum(~is_dve) - 1          # streamed sval tile index
    NDV = int(is_dve.sum())
    NSV = TCH - NDV
    NBL = K.shape[1]
    GCOLS_B = [int(ncalls[b]) * (cfg.CALL // 16) for b in range(NBL)]
    GOFF = np.concatenate([[0], np.cumsum(GCOLS_B)]).astype(int)
    return dict(K=K, TCH=TCH, Sb=Sb, ncalls=ncalls, lastvalid=lastvalid,
                is_dve=is_dve, is_act=is_act, dcol_of=dcol_of, svi_of=svi_of,
                NDV=NDV, NSV=max(NSV, 1), GOFF=GOFF)


def _core_layer_maps(sch, cellcnt, rel_src, ep, ev, cfg):
    """Build gather idx stream + per-chunk metadata for one (core, layer).
    rel_src: int64 block-relative row index per edge (cell-sorted order).
    Returns dict with gidx [128, GCOLS], drel/vval [128, NDV] f32,
    sval [128, NSV*128] bf16."""
    WN, CALL = cfg.WN, cfg.CALL
    K, TCH, Sb, ncalls = sch["K"], sch["TCH"], sch["Sb"], sch["ncalls"]
    is_dve, dcol_of, svi_of = sch["is_dve"], sch["dcol_of"], sch["svi_of"]
    NDV, NSV = sch["NDV"], sch["NSV"]
    NBL = K.shape[1]

    gstreams = [np.zeros(int(ncalls[b]) * CALL, np.int16) for b in range(NBL)]
    for b in range(NBL):
        if Sb[b] < ncalls[b] * CALL:
            gstreams[b][Sb[b]:] = -1  # tail of last call: skipped by DMA
    drel = np.full((max(NDV, 1), 128), -255.0, np.float32)
    vval = np.zeros((max(NDV, 1), 128), np.float32)
    sval = np.zeros((NSV, 128, 128), f8np)

    eoff = 0
    gcol = 0
    posb = np.zeros(NBL, np.int64)
    for w in range(WN):
        for b in range(NBL):
            L = int(cellcnt[w, b])
            kwb = int(K[w, b])
            if kwb == 0:
                assert L == 0
                continue
            sl = slice(eoff, eoff + L)
            st = int(posb[b])
            gstreams[b][st:st + L] = rel_src[sl].astype(np.int16)
            epc = ep[sl]
            evc = ev[sl]
            for kk in range(kwb):
                r0, r1 = kk * 128, min((kk + 1) * 128, L)
                gc = gcol + kk
                if is_dve[gc]:
                    dc = int(dcol_of[gc])
                    if r1 > r0:
                        drel[dc, :r1 - r0] = -epc[r0:r1]
                        vval[dc, :r1 - r0] = evc[r0:r1].astype(np.float32)
                else:
                    si = int(svi_of[gc])
                    if r1 > r0:
                        sval[si, np.arange(r1 - r0), epc[r0:r1]] = 1.0
            posb[b] += kwb * 128
            gcol += kwb
            eoff += L
    assert gcol == TCH

    gparts = []
    for b in range(NBL):
        arr = gstreams[b].reshape(-1, 16).T  # [16, Sb_pad/16]
        gparts.append(arr)
    gidx16 = np.concatenate(gparts, axis=1)          # [16, GCOLS]
    gidx = np.tile(gidx16, (8, 1)).astype(np.int16)  # [128, GCOLS]
    return dict(
        gidx=gidx,
        drel=np.ascontiguousarray(drel.T), vval=np.ascontiguousarray(vval.T),
        sval=np.ascontiguousarray(sval.transpose(1, 0, 2).reshape(128, -1)),
    ), eoff


def prep(x, edge_index, cfg=CFG):
    """Host-side sharding/schedule. Returns (schedule, per-core input maps)."""
    C, NB, WN, NSH, BS = cfg.C, cfg.NB, cfg.WN, cfg.NSH, cfg.BS
    SLOTS, G = cfg.SLOTS, cfg.G
    WCH = cfg.WCH
    src = np.asarray(edge_index[0]).astype(np.int64)
    dst = np.asarray(edge_index[1]).astype(np.int64)

    deg = np.bincount(dst, minlength=cfg.N).astype(np.float64)
    invdeg = (1.0 / np.maximum(deg, 1.0)).astype(np.float32)
    vedge_all = invdeg[dst].astype(bfnp)

    ecore = dst // NSH
    eblock1 = src // BS

    # --- per-core balance: node-local id -> (window, pos)
    win_of = np.zeros(cfg.N, np.int64)   # window within core
    pos_of = np.zeros(cfg.N, np.int64)   # slot within window
    counts1 = np.zeros((C, WN, NB), np.int64)
    for c in range(C):
        lo = c * NSH
        dnb = np.zeros((NSH, NB), np.int64)
        emask = ecore == c
        np.add.at(dnb, (dst[emask] - lo, eblock1[emask]), 1)
        assign, binsum = _balance_core(dnb, WN)
        # order bins by descending per-block chunk tuple so heavy cells align
        # at the same window index across cores
        kt = np.ceil(binsum / 128).astype(np.int64)
        key = [tuple(-kt[j]) + tuple(-binsum[j]) for j in range(WN)]
        order = sorted(range(WN), key=lambda j: key[j])
        rank = np.empty(WN, np.int64)
        rank[order] = np.arange(WN)
        w = rank[assign]
        win_of[lo:lo + NSH] = w
        # position within window: stable by node id
        order2 = np.lexsort((np.arange(NSH), w))
        pos = np.zeros(NSH, np.int64)
        pcount = np.zeros(WN, np.int64)
        for m in order2:
            pos[m] = pcount[w[m]]
            pcount[w[m]] += 1
        pos_of[lo:lo + NSH] = pos
        cnt = np.zeros((WN, NB), np.int64)
        np.add.at(cnt, (w[dst[emask] - lo], eblock1[emask]), 1)
        counts1[c] = cnt

    # layer-2 geometry: hfull is chunk-major: for AllGather chunk g
    # (windows [WCH[g], WCH[g+1])), node row = cbase[g] + core*crows[g] +
    # (win - WCH[g])*128 + pos; gather block g covers rows
    # [cbase[g], cbase[g+1]).
    if G == 1:
        # core-major hfull: blocks of 2 cores (2*NSH == BS) share layer-1's
        # cell structure exactly
        g2_of = (np.arange(cfg.N) // NSH) * SLOTS + win_of * 128 + pos_of
        eblock2 = eblock1
        cbase = np.arange(NB + 1, dtype=np.int64) * 2 * SLOTS
        counts2 = counts1
    else:
        crows = np.array([(WCH[g + 1] - WCH[g]) * 128 for g in range(G)], np.int64)
        cbase = np.concatenate([[0], np.cumsum(crows * C)]).astype(np.int64)
        for g in range(G):
            assert crows[g] * C <= 32767, (g, crows[g] * C)
        gchunk_of_w = np.zeros(WN, np.int64)
        for g in range(G):
            gchunk_of_w[WCH[g]:WCH[g + 1]] = g
        wch_arr = np.asarray(WCH)
        vg = gchunk_of_w[win_of]
        g2_of = (cbase[vg] + (np.arange(cfg.N) // NSH) * crows[vg]
                 + (win_of - wch_arr[vg]) * 128 + pos_of)
        eblock2 = vg[src]
        counts2 = np.zeros((C, WN, G), np.int64)
        for c in range(C):
            emask = ecore == c
            np.add.at(counts2[c], (win_of[dst[emask]], eblock2[emask]), 1)

    sch1 = _layer_sched(counts1, cfg)
    sch2 = _layer_sched(counts2, cfg)

    ewin = win_of[dst]
    epos = pos_of[dst]

    in_maps = []
    for c in range(C):
        lo = c * NSH
        emask = ecore == c
        es = src[emask]
        ew, eb1, eb2 = ewin[emask], eblock1[emask], eblock2[emask]
        ep = epos[emask]
        ev = vedge_all[emask]
        eg2 = g2_of[es]

        # layer 1: cells (w, src//BS), sorted by src within cell
        o1 = np.lexsort((es, eb1, ew))
        m1, ne1 = _core_layer_maps(
            sch1, counts1[c], (es - eb1 * BS)[o1], ep[o1], ev[o1], cfg)
        assert ne1 == es.shape[0]
        # layer 2: cells (w, chunk(src)), sorted by hfull row within cell
        rel2 = eg2 - cbase[eb2]
        o2 = np.lexsort((rel2, eb2, ew))
        m2, ne2 = _core_layer_maps(
            sch2, counts2[c], rel2[o2], ep[o2], ev[o2], cfg)
        assert ne2 == es.shape[0]

        # slot s = win*128+pos of node-local rows; dummy slots unused
        msk = np.arange(cfg.N)[lo:lo + NSH]
        sl_idx = win_of[msk] * 128 + pos_of[msk]
        sl_nodes = np.full(cfg.SLOTS, -1, np.int64)
        sl_nodes[sl_idx] = np.arange(NSH)

        # xT in slot order
        xT = np.zeros((cfg.D, cfg.SLOTS), bfnp)
        xT[:, sl_idx] = np.asarray(x[lo:lo + NSH]).astype(bfnp).T
        invsl = np.ones((cfg.SLOTS,), np.float32)
        invsl[sl_idx] = invdeg[lo:lo + NSH]
        bcast = np.ascontiguousarray(
            np.broadcast_to(invsl.astype(bfnp), (128, cfg.SLOTS)))

        in_maps.append(dict(
            gidx1=m1["gidx"], dstrel1=m1["drel"], vval1=m1["vval"], sval1=m1["sval"],
            gidx2=m2["gidx"], dstrel2=m2["drel"], vval2=m2["vval"], sval2=m2["sval"],
            xT=np.ascontiguousarray(xT), bcast=bcast,
            slot_nodes=sl_nodes,                   # host-only
        ))

    sched = dict(sch1=sch1, sch2=sch2, cbase=cbase,
                 TCH=sch1["TCH"] + sch2["TCH"])
    return sched, in_maps


# ---------------------------------------------------------------- program


def build(cfg, sched):
    sch = {1: sched["sch1"], 2: sched["sch2"]}
    cbase = sched["cbase"]
    C, D, NB, WN, BS = cfg.C, cfg.D, cfg.NB, cfg.WN, cfg.BS
    CALL, CALLCH, SLOTS, G = cfg.CALL, cfg.CALLCH, cfg.SLOTS, cfg.G
    WCH = cfg.WCH

    nc = bacc.Bacc(None, num_devices=C, num_swdge_queues=4,
                   dynamic_dma_scratch_size=24576)
    x_d = nc.dram_tensor("xbf", [cfg.N, D], BF, kind="ExternalInput")
    xT_d = nc.dram_tensor("xT", [D, SLOTS], BF, kind="ExternalInput")
    gidx_d, drel_d, vval_d, sval_d = {}, {}, {}, {}
    for L in (1, 2):
        s = sch[L]
        gidx_d[L] = nc.dram_tensor(f"gidx{L}", [128, int(s["GOFF"][-1])], I16,
                                   kind="ExternalInput")
        drel_d[L] = nc.dram_tensor(f"dstrel{L}", [128, max(s["NDV"], 1)], F32,
                                   kind="ExternalInput")
        vval_d[L] = nc.dram_tensor(f"vval{L}", [128, max(s["NDV"], 1)], F32,
                                   kind="ExternalInput")
        sval_d[L] = nc.dram_tensor(f"sval{L}", [128, s["NSV"] * 128], FP8,
                                   kind="ExternalInput")
    w_d = {}
    for nm in ("wlt1", "wrt1", "wlt2", "wrt2"):
        w_d[nm] = nc.dram_tensor(nm, [D, D], BF, kind="ExternalInput")
    b1_d = nc.dram_tensor("b1c", [D, 1], F32, kind="ExternalInput")
    b2_d = nc.dram_tensor("b2r", [1, D], F32, kind="ExternalInput")
    bc_d = nc.dram_tensor("bcast", [128, SLOTS], BF, kind="ExternalInput")
    out_d = nc.dram_tensor("out", [SLOTS, D], BF, kind="ExternalOutput")

    ident_d = nc.inline_tensor(np.eye(128, dtype=bfnp), "identc")
    iota_d = nc.inline_tensor(
        np.broadcast_to(np.arange(128, dtype=bfnp), (128, 128)).copy(), "iotac")
    ones_d = nc.inline_tensor(np.ones((1, 128), np.float32), "onesc")

    hsl_d = nc.dram_tensor("hslots", [SLOTS, D], BF)  # Internal, slot order
    hfull_d = nc.dram_tensor("hfull", [C * SLOTS, D], BF, addr_space="Shared")

    with tile.TileContext(nc) as tc, ExitStack() as ctx:
        const = ctx.enter_context(tc.tile_pool(name="const", bufs=1))
        meta = ctx.enter_context(tc.tile_pool(name="meta", bufs=1))
        gpool = ctx.enter_context(tc.tile_pool(name="gather", bufs=5))
        spool = ctx.enter_context(tc.tile_pool(name="sv", bufs=6))
        ohp = ctx.enter_context(tc.tile_pool(name="oh", bufs=1))
        dhp = ctx.enter_context(tc.tile_pool(name="dh", bufs=1))
        mwp = ctx.enter_context(tc.tile_pool(name="mw", bufs=6))
        htp = ctx.enter_context(tc.tile_pool(name="ht", bufs=1))
        xtp = ctx.enter_context(tc.tile_pool(name="xt", bufs=2))
        stgp = ctx.enter_context(tc.tile_pool(name="stg", bufs=2))
        ostgp = ctx.enter_context(tc.tile_pool(name="ostg", bufs=2))
        psA = ctx.enter_context(tc.tile_pool(name="psA", bufs=4, space="PSUM"))
        psB = ctx.enter_context(tc.tile_pool(name="psB", bufs=2, space="PSUM"))
        psT = ctx.enter_context(tc.tile_pool(name="psT", bufs=2, space="PSUM"))
        bcp = ctx.enter_context(tc.tile_pool(name="bc", bufs=1))

        def load(pool, dram, shape, dtype):
            t = pool.tile(shape, dtype, tag=dram.name)
            nc.sync.dma_start(t[:], dram[:])
            return t

        ident_s = load(const, ident_d, [128, 128], BF)
        iota_s = load(const, iota_d, [128, 128], BF)
        ones_s = load(const, ones_d, [1, 128], F32)
        w_s = {nm: load(const, w_d[nm], [D, D], BF) for nm in w_d}
        b1_s = load(const, b1_d, [D, 1], F32)
        b2_s = load(const, b2_d, [1, D], F32)

        drel_s = {L: load(meta, drel_d[L], [128, max(sch[L]["NDV"], 1)], F32)
                  for L in (1, 2)}
        vval_s = {L: load(meta, vval_d[L], [128, max(sch[L]["NDV"], 1)], F32)
                  for L in (1, 2)}
        gidx_s = {}
        for L in (1, 2):
            gidx_s[L] = meta.tile([128, int(sch[L]["GOFF"][-1])], I16,
                                  tag=f"gidx{L}", name=f"gidx{L}")

        def load_gidx(L):
            GOFF = sch[L]["GOFF"]
            for _b in range(sch[L]["K"].shape[1]):
                nc.sync.dma_start(gidx_s[L][:, int(GOFF[_b]):int(GOFF[_b + 1])],
                                  gidx_d[L][:, int(GOFF[_b]):int(GOFF[_b + 1])])

        load_gidx(1)
        # prefetch the first sval groups of layer 1 ahead of the bulk loads
        prefetched = {}
        for _si in range(2):
            _nch = min(8, sch[1]["NSV"] - _si * 8)
            _stt = spool.tile([128, 8, 128], FP8, tag="sv", name=f"sv1_{_si}")
            nc.sync.dma_start(_stt[:, :_nch, :],
                              sval_d[1][:, _si * 1024:_si * 1024 + _nch * 128])
            prefetched[_si] = _stt
        bc_s = load(bcp, bc_d, [128, SLOTS], BF)
        hT_s = htp.tile([D, SLOTS], BF, tag="hT")

        def run_layer(L, w_lo, w_hi):
            s = sch[L]
            K, ncalls, lastvalid = s["K"], s["ncalls"], s["lastvalid"]
            is_dve, dcol_of, svi_of, NSV = (s["is_dve"], s["dcol_of"],
                                            s["svi_of"], s["NSV"])
            GOFF = s["GOFF"]
            NBL = K.shape[1]
            src_d = x_d if L == 1 else hfull_d
            drl, vvl, gix = drel_s[L], vval_s[L], gidx_s[L]
            st_ = run_layer.state.setdefault(
                L, dict(posb=[0] * NBL, gt=[None] * NBL, gcol=0, stile=None,
                        stg=None, ostg=None, bct=None, xtt=None))
            posb, gt = st_["posb"], st_["gt"]
            SCW, OCW = cfg.SCW, cfg.OCW
            for w in range(w_lo, w_hi):
                nchunks_w = int(K[w].sum())
                psum_a = psA.tile([128, 128], F32, tag="agg", name=f"agg{L}_{w}")
                ci = 0
                for b in range(NBL):
                    for k in range(int(K[w, b])):
                        pos = posb[b]
                        call_i, col = divmod(pos, CALLCH)
                        if col == 0:
                            gt[b] = gpool.tile([128, CALLCH, 128], BF, tag=f"g{b}",
                                               name=f"g{b}_{L}_{call_i}")
                            nvalid = CALL if call_i < int(ncalls[b]) - 1 else int(lastvalid[b])
                            ioff = GOFF[b] + call_i * (CALL // 16)
                            if L == 1:
                                in_ap = src_d[b * BS:(b + 1) * BS, :]
                            else:
                                in_ap = src_d[int(cbase[b]):int(cbase[b + 1]), :]
                            nc.gpsimd.dma_gather(
                                out_ap=gt[b][:],
                                in_ap=in_ap,
                                idxs_ap=gix[:, ioff:ioff + CALL // 16],
                                num_idxs=CALL,
                                num_idxs_reg=nvalid,
                                elem_size=D,
                            )
                        gcol = st_["gcol"]
                        if is_dve[gcol]:
                            dc = int(dcol_of[gcol])
                            dd = dhp.tile([128, 128], BF, tag="dh",
                                          name=f"dh{L}_{gcol}")
                            nc.scalar.activation(
                                dd[:], iota_s[:],
                                mybir.ActivationFunctionType.Abs,
                                bias=drl[:, dc:dc + 1], scale=1.0)
                            S = ohp.tile([128, 128], BF, tag="oh",
                                         name=f"oh{L}_{gcol}")
                            nc.scalar.activation(
                                S[:], dd[:],
                                mybir.ActivationFunctionType.Relu,
                                bias=1.0, scale=-1.0)
                            rhs_ap = S[:]
                        else:
                            si, sc = divmod(int(svi_of[gcol]), 8)
                            if sc == 0:
                                if L == 1 and si in prefetched:
                                    st_["stile"] = prefetched.pop(si)
                                elif (L, si) in prefetched:
                                    st_["stile"] = prefetched.pop((L, si))
                                else:
                                    nch = min(8, NSV - si * 8)
                                    stt = spool.tile([128, 8, 128], FP8, tag="sv",
                                                     name=f"sv{L}_{si}")
                                    nc.sync.dma_start(
                                        stt[:, :nch, :],
                                        sval_d[L][:, si * 1024:si * 1024 + nch * 128])
                                    st_["stile"] = stt
                            rhs_ap = st_["stile"][:, sc, :]
                        nc.tensor.matmul(
                            out=psum_a[:], lhsT=gt[b][:, col, :], rhs=rhs_ap,
                            start=(ci == 0), stop=(ci == nchunks_w - 1),
                        )
                        st_["gcol"] += 1
                        posb[b] += 1
                        ci += 1
                m_s = mwp.tile([128, 128], BF, tag="mw", name=f"mw{L}_{w}")
                wsl = slice(w * 128, (w + 1) * 128)
                if nchunks_w:
                    nc.vector.tensor_tensor(out=m_s[:], in0=psum_a[:],
                                            in1=bc_s[:, wsl],
                                            op=mybir.AluOpType.mult)
                else:
                    nc.vector.memset(m_s[:], 0.0)
                if L == 1:
                    psum_h = psB.tile([128, 128], F32, tag="h", name=f"h{L}_{w}")
                    nc.tensor.matmul(out=psum_h[:], lhsT=w_s["wlt1"][:], rhs=m_s[:],
                                     start=True, stop=False)
                    xgi, xgo = divmod(w, 8)
                    if xgo == 0 or st_["xtt"] is None:
                        nxw = min(8, WN - xgi * 8)
                        xtt = xtp.tile([128, 8, 128], BF, tag="xT",
                                       name=f"xT{xgi}")
                        nc.sync.dma_start(
                            xtt[:, :nxw, :],
                            xT_d[:, xgi * 1024:xgi * 1024 + nxw * 128])
                        st_["xtt"] = xtt
                    nc.tensor.matmul(out=psum_h[:], lhsT=w_s["wrt1"][:],
                                     rhs=st_["xtt"][:, xgo, :], start=False,
                                     stop=True)
                    nc.scalar.activation(hT_s[:, wsl], psum_h[:],
                                         mybir.ActivationFunctionType.Identity,
                                         bias=b1_s[:, 0:1], scale=1.0)
                    psum_t = psT.tile([128, 128], BF, tag="tr", name=f"tr{w}")
                    nc.tensor.transpose(psum_t[:], hT_s[:, wsl], ident_s[:])
                    wi = (w - w_lo) % SCW
                    if wi == 0:
                        st_["stg"] = stgp.tile([128, SCW, 128], BF, tag="stg",
                                               name=f"stg{w}")
                    nc.scalar.copy(st_["stg"][:, wi, :], psum_t[:])
                    if wi == SCW - 1 or w == w_hi - 1:
                        used = wi + 1
                        w0 = w - wi
                        hap = hsl_d[:].rearrange("(w p) f -> p w f", p=128)
                        nc.sync.dma_start(hap[:, w0:w0 + used, :],
                                          st_["stg"][:, :used, :])
                else:
                    psum_h = psB.tile([128, 128], F32, tag="h", name=f"h{L}_{w}")
                    nc.tensor.matmul(out=psum_h[:], lhsT=m_s[:], rhs=w_s["wlt2"][:],
                                     start=True, stop=False)
                    nc.tensor.matmul(out=psum_h[:], lhsT=hT_s[:, wsl],
                                     rhs=w_s["wrt2"][:], start=False, stop=False)
                    nc.tensor.matmul(out=psum_h[:], lhsT=ones_s[0:1, :],
                                     rhs=b2_s[0:1, :], start=False, stop=True)
                    wi = w % OCW
                    if wi == 0:
                        st_["ostg"] = ostgp.tile([128, OCW, 128], BF, tag="ostg",
                                                 name=f"ostg{w}")
                    nc.scalar.copy(st_["ostg"][:, wi, :], psum_h[:])
                    if wi == OCW - 1 or w == WN - 1:
                        used = wi + 1
                        w0 = w - wi
                        oap = out_d[:].rearrange("(w p) f -> p w f", p=128)
                        nc.sync.dma_start(oap[:, w0:w0 + used, :],
                                          st_["ostg"][:, :used, :])

        run_layer.state = {}
        # layer 1 in window chunks; AllGather each chunk as soon as its
        # windows are stored (overlaps the collective with remaining compute)
        for g in range(G):
            run_layer(1, WCH[g], WCH[g + 1])
            if g == G - 1:
                load_gidx(2)
                for _si in range(4):
                    _nch = min(8, sch[2]["NSV"] - _si * 8)
                    _stt = spool.tile([128, 8, 128], FP8, tag="sv",
                                      name=f"sv2_{_si}")
                    nc.sync.dma_start(
                        _stt[:, :_nch, :],
                        sval_d[2][:, _si * 1024:_si * 1024 + _nch * 128])
                    prefetched[(2, _si)] = _stt
            r0, r1 = WCH[g] * 128, WCH[g + 1] * 128
            out_lo = 0 if G == 1 else int(cbase[g])
            out_hi = C * SLOTS if G == 1 else int(cbase[g + 1])
            nc.gpsimd.collective_compute(
                "AllGather", mybir.AluOpType.bypass,
                replica_groups=[list(range(C))],
                ins=[hsl_d[r0:r1, :]],
                outs=[hfull_d[out_lo:out_hi, :]],
            )
        run_layer(2, 0, WN)

    # spread SWDGE gather descriptor generation across the 4 SWDGE queues
    # (parallel Q7 pairs). Tile assigned DMASW lanes round-robin in scheduled
    # order; keep sem-lane <-> queue binding consistent by deriving the queue
    # from the lane (lane % 4).
    from concourse.tile_sem_assignment import PROC_NAME_TO_IDX
    dmasw0 = PROC_NAME_TO_IDX["DMASW0"]
    for inst in nc.inst_map.values():
        if isinstance(inst, (mybir.InstDMAGatherAnt, mybir.InstDMAScatterAddAnt)):
            proc = getattr(inst, "bass_scheduled_proc", None)
            if proc is not None and dmasw0 <= proc < dmasw0 + 8:
                inst.queue_num = (proc - dmasw0) % 4

    nc.compile()
    return nc


# ---------------------------------------------------------------- kernel


def kernel(**inputs):
    cfg = CFG
    x = np.asarray(inputs["x"], np.float32)
    ei = np.asarray(inputs["edge_index"])
    sched, in_maps = prep(x, ei, cfg)
    nc = build(cfg, sched)

    x_bf = x.astype(bfnp)
    shared = dict(
        xbf=x_bf,
        wlt1=np.ascontiguousarray(np.asarray(inputs["Wl1"], np.float32).T.astype(bfnp)),
        wrt1=np.ascontiguousarray(np.asarray(inputs["Wr1"], np.float32).T.astype(bfnp)),
        wlt2=np.ascontiguousarray(np.asarray(inputs["Wl2"], np.float32).T.astype(bfnp)),
        wrt2=np.ascontiguousarray(np.asarray(inputs["Wr2"], np.float32).T.astype(bfnp)),
        b1c=np.asarray(inputs["b1"], np.float32).reshape(cfg.D, 1).copy(),
        b2r=np.asarray(inputs["b2"], np.float32).reshape(1, cfg.D).copy(),
    )
    slot_nodes = [m.pop("slot_nodes") for m in in_maps]
    run_maps = [dict(shared, **{k: v for k, v in m.items()}) for m in in_maps]

    res = None
    last_err = None
    for attempt in range(3):
        try:
            res = run_bass_kernel_spmd(nc, run_maps, core_ids=list(range(cfg.C)))
            break
        except Exception as e:  # transient device wedge: retry
            last_err = e
            import time
            time.sleep(10)
    if res is None:
        raise last_err
    out = np.empty((cfg.N, cfg.D), np.float32)
    for c in range(cfg.C):
        oc = res.results[c]["out"]
        sn = slot_nodes[c]
        real = sn >= 0
        out[c * cfg.NSH + sn[real]] = oc[real]
    return out


if __name__ == "__main__":
    d = np.load("/tmp/inputs.npz")
    ins = {k: d[k] for k in ("x", "edge_index", "Wl1", "Wr1", "b1", "Wl2", "Wr2", "b2")}
    got = kernel(**ins)
    exp = d["expected"]
    err = np.abs(got - exp).max() / np.abs(exp).max()
    print("Relative error:", err)
